# revision 24
# baseline (speedup 1.0000x reference)
"""AttentionBlock (GroupNorm + 8-head self-attention + proj + residual) on 8 trn2 cores.

Sharding: data-parallel over batch (16 batches -> 2 per core), no collectives.

Per-core device program (per batch):
  - GroupNorm(32, 512): bn_stats per 128-channel tile -> per-channel [mean, E[x^2]]
    -> cross-partition group reduce via a (128,128) group-indicator fp32 matmul
    -> per-channel scale/bias -> hn (bf16).
  - QKV 1x1 conv as matmuls (bf16): q,k produced in (channel, pixel) layout;
    v produced in (pixel, channel) layout, padded with a ones column per head.
  - Attention head-pair j=(2j, 2j+1): the two heads sit at partition offsets
    0/64 of the same qkT chunk, so their K=64 score matmuls occupy disjoint
    PE row-groups (tile_position auto-derived from base partition) and run
    concurrently. exp on ScalarE straight from PSUM (scores bounded ~6.5 ->
    no max subtraction); ScalarE is the kernel bottleneck, so qkv/v/proj
    matmul work is interleaved between score blocks to keep it fed.
  - AV matmul with the ones column producing the softmax denominator as psum
    row 64 (two 1-bank halves for pipelining). Denominator reciprocal ->
    broadcast over 64 partitions via a K=1 ones matmul -> normalize o.
  - proj matmul + (bias + residual) fused in one DVE op -> DMA out.
"""

import numpy as np
import ml_dtypes

import concourse.bass as bass
import concourse.tile as tile
from concourse import bacc, mybir

B, C, HH, WW = 16, 512, 32, 32
N = HH * WW          # 1024 pixels
NH, HD = 8, 64       # heads, head dim
NG, GS = 32, 16      # groups, channels per group
NCORES = 8
BPC = B // NCORES    # batches per core
NT = C // 128        # channel tiles of 128
EPS = 1e-5
SCALE = HD ** -0.5

F32 = mybir.dt.float32
BF16 = mybir.dt.bfloat16


def build_program(qk_bufs=2, out_bufs=3):
    nc = bacc.Bacc(None, target_bir_lowering=False, debug=False)

    x_d = nc.declare_dram_parameter("x", [BPC, 128, NT, N], F32, isOutput=False)
    wqkv_d = nc.declare_dram_parameter("wqkv", [128, NT, 3 * C], BF16, isOutput=False)
    wp_d = nc.declare_dram_parameter("wp", [128, NT, C], BF16, isOutput=False)
    qkvb_d = nc.declare_dram_parameter("qkvb", [128, 2 * NT], F32, isOutput=False)
    vbias_d = nc.declare_dram_parameter("vbias", [128, NH * 65], F32, isOutput=False)
    pb_d = nc.declare_dram_parameter("pb", [128, NT], F32, isOutput=False)
    nw_d = nc.declare_dram_parameter("nw", [128, NT], F32, isOutput=False)
    nb_d = nc.declare_dram_parameter("nb", [128, NT], F32, isOutput=False)
    gsel_d = nc.declare_dram_parameter("gsel", [128, 128], F32, isOutput=False)
    out_d = nc.declare_dram_parameter("out", [BPC, 128, NT, N], F32, isOutput=True)

    with tile.TileContext(nc) as tc:
        with (
            tc.tile_pool(name="consts", bufs=1) as consts,
            tc.tile_pool(name="xpool", bufs=2) as xpool,
            tc.tile_pool(name="hnpool", bufs=2) as hnpool,
            tc.tile_pool(name="qkpool", bufs=qk_bufs) as qkpool,
            tc.tile_pool(name="vpool", bufs=2) as vpool,
            tc.tile_pool(name="epool", bufs=1) as epool,
            tc.tile_pool(name="opool", bufs=2) as opool,
            tc.tile_pool(name="dpool", bufs=4) as dpool,
            tc.tile_pool(name="outpool", bufs=out_bufs) as outpool,
            tc.tile_pool(name="spool", bufs=2) as spool,
            tc.tile_pool(name="psum", bufs=3, space="PSUM") as psum,
        ):
            # ---- constants / weights ----
            wqkv_sb = consts.tile([128, NT, 3 * C], BF16)
            nc.sync.dma_start(out=wqkv_sb, in_=wqkv_d[:])
            wp_sb = consts.tile([128, NT, C], BF16)
            nc.sync.dma_start(out=wp_sb, in_=wp_d[:])
            qkvb_sb = consts.tile([128, 2 * NT], F32)
            nc.sync.dma_start(out=qkvb_sb, in_=qkvb_d[:])
            vbias_sb = consts.tile([128, NH * 65], F32)
            nc.sync.dma_start(out=vbias_sb, in_=vbias_d[:])
            pb_sb = consts.tile([128, NT], F32)
            nc.sync.dma_start(out=pb_sb, in_=pb_d[:])
            nw_sb = consts.tile([128, NT], F32)
            nc.sync.dma_start(out=nw_sb, in_=nw_d[:])
            nb_sb = consts.tile([128, NT], F32)
            nc.sync.dma_start(out=nb_sb, in_=nb_d[:])
            gsel_sb = consts.tile([128, 128], F32)
            nc.sync.dma_start(out=gsel_sb, in_=gsel_d[:])
            eps_sb = consts.tile([128, 1], F32)
            nc.vector.memset(eps_sb, EPS)
            ones64 = consts.tile([1, 64], BF16)
            nc.vector.memset(ones64, 1.0)

            BNS = nc.vector.BN_STATS_DIM   # 6
            BNA = nc.vector.BN_AGGR_DIM    # 2

            # ---- groupnorm for both batches (all sqrt ACT ops before any exp) ----
            state = {}
            for b in range(BPC):
                x_sb = xpool.tile([128, NT, N], F32)
                nc.sync.dma_start(out=x_sb, in_=x_d[b])

                stats4 = spool.tile([128, 2 * NT], F32)
                for t in range(NT):
                    bnstat = spool.tile([128, 2, BNS], F32)
                    xv = x_sb[:, t, :].rearrange("p (s n) -> p s n", s=2)
                    for s in range(2):
                        nc.vector.bn_stats(out=bnstat[:, s, :], in_=xv[:, s, :])
                    mv = spool.tile([128, BNA], F32)
                    nc.vector.bn_aggr(out=mv, in_=bnstat)
                    nc.vector.tensor_copy(out=stats4[:, t : t + 1], in_=mv[:, 0:1])
                    nc.vector.scalar_tensor_tensor(
                        out=stats4[:, NT + t : NT + t + 1],
                        in0=mv[:, 0:1],
                        scalar=mv[:, 0:1],
                        in1=mv[:, 1:2],
                        op0=mybir.AluOpType.mult,
                        op1=mybir.AluOpType.add,
                    )

                pst = psum.tile([128, 2 * NT], F32, tag="s")
                nc.tensor.matmul(pst[:], gsel_sb[:], stats4[:], start=True, stop=True)

                mean4 = spool.tile([128, NT], F32)
                nc.vector.tensor_scalar_mul(out=mean4, in0=pst[:, 0:NT], scalar1=1.0 / GS)
                msq4 = spool.tile([128, NT], F32)
                nc.vector.tensor_mul(out=msq4, in0=mean4, in1=mean4)
                var4 = spool.tile([128, NT], F32)
                nc.vector.scalar_tensor_tensor(
                    out=var4,
                    in0=pst[:, NT : 2 * NT],
                    scalar=1.0 / GS,
                    in1=msq4,
                    op0=mybir.AluOpType.mult,
                    op1=mybir.AluOpType.subtract,
                )
                std4 = spool.tile([128, NT], F32)
                nc.scalar.activation(
                    out=std4, in_=var4, func=mybir.ActivationFunctionType.Sqrt,
                    bias=eps_sb[:, 0:1], scale=1.0,
                )
                rstd4 = spool.tile([128, NT], F32)
                nc.vector.reciprocal(out=rstd4, in_=std4)
                a4 = spool.tile([128, NT], F32)
                nc.vector.tensor_mul(out=a4, in0=rstd4, in1=nw_sb)
                mb4 = spool.tile([128, NT], F32)
                nc.vector.tensor_mul(out=mb4, in0=mean4, in1=a4)
                b4 = spool.tile([128, NT], F32)
                nc.vector.tensor_sub(out=b4, in0=nb_sb, in1=mb4)

                hn = hnpool.tile([128, NT, N], BF16)
                for t in range(NT):
                    nc.vector.tensor_scalar(
                        out=hn[:, t, :],
                        in0=x_sb[:, t, :],
                        scalar1=a4[:, t : t + 1],
                        scalar2=b4[:, t : t + 1],
                        op0=mybir.AluOpType.mult,
                        op1=mybir.AluOpType.add,
                    )
                state[b] = {"x": x_sb, "hn": hn}

            # ---- emission helpers (PE queue is in-order: keep ScalarE fed) ----
            def emit_qk_chunk(st, j):
                """q chunk j and k chunk j of qkT (rows j*128 / (NT+j)*128)."""
                hn = st["hn"]
                for rb in (j, NT + j):
                    ps = psum.tile([128, N], F32, tag="s", name="ps_qk")
                    for half in range(2):
                        for kc in range(NT):
                            nc.tensor.matmul(
                                ps[:, half * 512 : (half + 1) * 512],
                                wqkv_sb[:, kc, rb * 128 : (rb + 1) * 128],
                                hn[:, kc, half * 512 : (half + 1) * 512],
                                start=(kc == 0),
                                stop=(kc == NT - 1),
                            )
                    nc.vector.tensor_scalar_add(
                        out=st["qkT"][:, rb, :], in0=ps[:],
                        scalar1=qkvb_sb[:, rb : rb + 1],
                    )

            def emit_v(st):
                hn = st["hn"]
                v_pad = st["v_pad"]
                for mbp in range(4):
                    psv = psum.tile([128, N], F32, tag="s", name="psv")
                    for half in range(2):
                        mb = 2 * mbp + half
                        for kc in range(NT):
                            nc.tensor.matmul(
                                psv[:, half * 512 : (half + 1) * 512],
                                hn[:, kc, mb * 128 : (mb + 1) * 128],
                                wqkv_sb[:, kc, 2 * C : 3 * C],
                                start=(kc == 0),
                                stop=(kc == NT - 1),
                            )
                        nc.vector.tensor_tensor(
                            out=v_pad[:, mb, :].rearrange("p (h c) -> p h c", c=65)[
                                :, :, 0:64
                            ],
                            in0=psv[:, half * 512 : (half + 1) * 512].rearrange(
                                "p (h c) -> p h c", c=64
                            ),
                            in1=vbias_sb.rearrange("p (h c) -> p h c", c=65)[:, :, 0:64],
                            op=mybir.AluOpType.add,
                        )

            def emit_scores_exp(st, j):
                """Concurrent K=64 score matmuls for head pair (2j, 2j+1)."""
                qkT = st["qkT"]
                pair = []
                for i in range(2):
                    h = 2 * j + i
                    poff = (h % 2) * 64
                    qT_h = qkT[poff : poff + 64, h // 2, :]
                    kT_h = qkT[poff : poff + 64, NT + h // 2, :]
                    expT = epool.tile([128, 8, N], BF16, name="expT", tag=f"expT{i}")
                    pair.append((qT_h, kT_h, expT))
                for mb in range(8):
                    pss = [
                        psum.tile([128, N], F32, tag="s", name="ps_s")
                        for _ in range(2)
                    ]
                    for half in range(2):
                        for i in range(2):
                            qT_h, kT_h, _ = pair[i]
                            nc.tensor.matmul(
                                pss[i][:, half * 512 : (half + 1) * 512],
                                kT_h[:, mb * 128 : (mb + 1) * 128],
                                qT_h[:, half * 512 : (half + 1) * 512],
                                start=True,
                                stop=True,
                            )
                    for i in range(2):
                        nc.scalar.activation(
                            out=pair[i][2][:, mb, :], in_=pss[i][:],
                            func=mybir.ActivationFunctionType.Exp, scale=SCALE,
                        )
                return [p[2] for p in pair]

            def emit_av_norm(st, j, expTs):
                v_pad = st["v_pad"]
                o_sb = st["o_sb"]
                for i in range(2):
                    h = 2 * j + i
                    poff = (h % 2) * 64
                    expT = expTs[i]
                    recips = []
                    for half in range(2):
                        po = psum.tile([65, 512], F32, tag="o", bufs=2, name="po")
                        for mb in range(8):
                            nc.tensor.matmul(
                                po[:],
                                v_pad[:, mb, h * 65 : (h + 1) * 65],
                                expT[:, mb, half * 512 : (half + 1) * 512],
                                start=(mb == 0),
                                stop=(mb == 7),
                            )
                        nc.vector.tensor_copy(
                            out=o_sb[
                                poff : poff + 64, h // 2, half * 512 : (half + 1) * 512
                            ],
                            in_=po[0:64, :],
                        )
                        recip_h = dpool.tile([1, 512], BF16, name="recip_h")
                        with nc.allow_low_precision(reason="softmax denom recip bf16"):
                            nc.vector.reciprocal(out=recip_h, in_=po[64:65, :])
                        recips.append(recip_h)
                    pbc = psum.tile([64, N], F32, tag="s", name="pbc")
                    for half in range(2):
                        nc.tensor.matmul(
                            pbc[:, half * 512 : (half + 1) * 512],
                            ones64[:],
                            recips[half][:],
                            start=True,
                            stop=True,
                        )
                    nc.vector.tensor_mul(
                        out=o_sb[poff : poff + 64, h // 2, :],
                        in0=o_sb[poff : poff + 64, h // 2, :],
                        in1=pbc[:],
                    )

            def emit_proj_rb(st, b, rb):
                o_sb = st["o_sb"]
                pp = psum.tile([128, N], F32, tag="s", name="pp")
                for half in range(2):
                    for kc in range(NT):
                        nc.tensor.matmul(
                            pp[:, half * 512 : (half + 1) * 512],
                            wp_sb[:, kc, rb * 128 : (rb + 1) * 128],
                            o_sb[:, kc, half * 512 : (half + 1) * 512],
                            start=(kc == 0),
                            stop=(kc == NT - 1),
                        )
                out_sb = outpool.tile([128, N], F32, name="out_sb")
                nc.vector.scalar_tensor_tensor(
                    out=out_sb,
                    in0=pp[:],
                    scalar=pb_sb[:, rb : rb + 1],
                    in1=st["x"][:, rb, :],
                    op0=mybir.AluOpType.add,
                    op1=mybir.AluOpType.add,
                )
                nc.sync.dma_start(out=out_d[b, :, rb, :], in_=out_sb[:])

            # ---- main interleaved schedule ----
            # per batch: iter j emits scores/exp(j), then filler PE work, then
            # AV(j) (which trails the exp stream).  Fillers: v + next qk chunk
            # at j=0, later qk chunks / prev-batch proj afterwards.
            for b in range(BPC):
                st = state[b]
                if "qkT" not in st:
                    st["qkT"] = qkpool.tile([128, 2 * NT, N], BF16, name="qkT")
                st["v_pad"] = vpool.tile([128, 8, NH * 65], BF16, name="v_pad")
                ones_view = st["v_pad"].rearrange("p m (h c) -> p m h c", c=65)[
                    :, :, :, 64:65
                ]
                nc.vector.memset(ones_view, 1.0)
                st["o_sb"] = opool.tile([128, NT, N], BF16, name="o_sb")

                if b == 0:
                    emit_qk_chunk(st, 0)  # batch 1's chunk 0 is a filler below

                for j in range(4):
                    expTs = emit_scores_exp(st, j)
                    # filler PE work while ScalarE chews on the exps
                    if j == 0:
                        emit_v(st)
                        emit_qk_chunk(st, 1)
                    elif j == 1:
                        emit_qk_chunk(st, 2)
                    elif j == 2:
                        emit_qk_chunk(st, 3)
                    else:
                        if b + 1 < BPC:
                            nxt = state[b + 1]
                            nxt["qkT"] = qkpool.tile(
                                [128, 2 * NT, N], BF16, name="qkT"
                            )
                            emit_qk_chunk(nxt, 0)
                        else:
                            for rb in range(NT):
                                emit_proj_rb(state[b - 1], b - 1, rb)
                    emit_av_norm(st, j, expTs)

            # tail: last batch's proj (prev batch's proj was a filler above)
            for rb in range(NT):
                emit_proj_rb(state[BPC - 1], BPC - 1, rb)

    nc.finalize()
    return nc


_PROGRAM = None


def _get_program():
    global _PROGRAM
    if _PROGRAM is None:
        try:
            _PROGRAM = build_program(qk_bufs=2, out_bufs=3)
        except Exception as e:
            import sys
            print(f"build fallback (qk_bufs=1): {type(e).__name__}", file=sys.stderr)
            _PROGRAM = build_program(qk_bufs=1, out_bufs=2)
    return _PROGRAM


def _prep_inputs(x, norm_w, norm_b, qkv_w, qkv_b, proj_w, proj_b):
    x = np.asarray(x, np.float32)
    xs = np.ascontiguousarray(
        x.reshape(B, NT, 128, N).transpose(0, 2, 1, 3)
    )  # (B, 128, NT, N)

    wqkvT = np.asarray(qkv_w, np.float32).T  # (C, 3C)
    wqkv = np.ascontiguousarray(
        wqkvT.reshape(NT, 128, 3 * C).transpose(1, 0, 2)
    ).astype(ml_dtypes.bfloat16)
    wpT = np.asarray(proj_w, np.float32).T
    wp = np.ascontiguousarray(wpT.reshape(NT, 128, C).transpose(1, 0, 2)).astype(
        ml_dtypes.bfloat16
    )

    qkv_b = np.asarray(qkv_b, np.float32)
    qkvb8 = np.ascontiguousarray(qkv_b[: 2 * C].reshape(2 * NT, 128).T)  # (128, 8)
    vb = np.zeros((NH, 65), np.float32)
    vb[:, :64] = qkv_b[2 * C :].reshape(NH, 64)
    vbias = np.ascontiguousarray(
        np.broadcast_to(vb.reshape(1, NH * 65), (128, NH * 65))
    )
    pb4 = np.ascontiguousarray(np.asarray(proj_b, np.float32).reshape(NT, 128).T)
    nw4 = np.ascontiguousarray(np.asarray(norm_w, np.float32).reshape(NT, 128).T)
    nb4 = np.ascontiguousarray(np.asarray(norm_b, np.float32).reshape(NT, 128).T)

    idx = np.arange(128)
    gsel = (idx[:, None] // GS == idx[None, :] // GS).astype(np.float32)

    shared = {
        "wqkv": wqkv, "wp": wp, "qkvb": qkvb8, "vbias": vbias, "pb": pb4,
        "nw": nw4, "nb": nb4, "gsel": gsel,
    }
    in_maps = [
        {"x": np.ascontiguousarray(xs[c * BPC : (c + 1) * BPC]), **shared}
        for c in range(NCORES)
    ]
    return in_maps


def _assemble(results):
    outs = np.concatenate(
        [results[c]["out"] for c in range(NCORES)], axis=0
    )  # (B, 128, NT, N)
    return np.ascontiguousarray(
        outs.transpose(0, 2, 1, 3).reshape(B, C, HH, WW)
    ).astype(np.float32)


def kernel(x, norm_w, norm_b, qkv_w, qkv_b, proj_w, proj_b, _trace=False):
    from concourse.bass_utils import run_bass_kernel_spmd

    nc = _get_program()
    in_maps = _prep_inputs(x, norm_w, norm_b, qkv_w, qkv_b, proj_w, proj_b)
    res = run_bass_kernel_spmd(nc, in_maps, list(range(NCORES)), trace=_trace)
    out = _assemble(res.results)
    if _trace:
        return out, res
    return out


# revision 27
# speedup vs baseline: 1.1853x; 1.1853x over previous
"""AttentionBlock (GroupNorm + 8-head self-attention + proj + residual) on 8 trn2 cores.

Sharding: data-parallel over batch (16 batches -> 2 per core), no collectives.

Per-core device program (per batch):
  - GroupNorm(32, 512): bn_stats per 128-channel tile -> per-channel [mean, E[x^2]]
    -> cross-partition group reduce via a (128,128) group-indicator fp32 matmul
    -> per-channel scale/bias -> hn (bf16).
  - QKV 1x1 conv as matmuls (bf16): q,k produced in (channel, pixel) layout;
    v produced in (pixel, channel) layout, padded with a ones column per head.
  - Attention head-pair j=(2j, 2j+1): the two heads sit at partition offsets
    0/64 of the same qkT chunk, so their K=64 score matmuls occupy disjoint
    PE row-groups (tile_position auto-derived from base partition) and run
    concurrently. exp on ScalarE straight from PSUM (scores bounded ~6.5 ->
    no max subtraction); ScalarE is the kernel bottleneck, so qkv/v/proj
    matmul work is interleaved between score blocks to keep it fed.
  - AV matmul with the ones column producing the softmax denominator as psum
    row 64 (two 1-bank halves for pipelining). Denominator reciprocal ->
    broadcast over 64 partitions via a K=1 ones matmul -> normalize o.
  - proj matmul + (bias + residual) fused in one DVE op -> DMA out.
"""

import numpy as np
import ml_dtypes

import concourse.bass as bass
import concourse.tile as tile
from concourse import bacc, mybir

B, C, HH, WW = 16, 512, 32, 32
N = HH * WW          # 1024 pixels
NH, HD = 8, 64       # heads, head dim
NG, GS = 32, 16      # groups, channels per group
NCORES = 8
BPC = B // NCORES    # batches per core
NT = C // 128        # channel tiles of 128
EPS = 1e-5
SCALE = HD ** -0.5

F32 = mybir.dt.float32
BF16 = mybir.dt.bfloat16


def build_program(qk_bufs=1, out_bufs=2):
    nc = bacc.Bacc(None, target_bir_lowering=False, debug=False)

    x_d = nc.declare_dram_parameter("x", [BPC, 128, NT, N], F32, isOutput=False)
    wqkv_d = nc.declare_dram_parameter("wqkv", [128, NT, 3 * C], BF16, isOutput=False)
    wp_d = nc.declare_dram_parameter("wp", [128, NT, C], BF16, isOutput=False)
    qkvb_d = nc.declare_dram_parameter("qkvb", [128, 2 * NT], F32, isOutput=False)
    vbias_d = nc.declare_dram_parameter("vbias", [128, NH * 65], BF16, isOutput=False)
    pb_d = nc.declare_dram_parameter("pb", [128, NT], F32, isOutput=False)
    nw_d = nc.declare_dram_parameter("nw", [128, NT], F32, isOutput=False)
    nb_d = nc.declare_dram_parameter("nb", [128, NT], F32, isOutput=False)
    gsel_d = nc.declare_dram_parameter("gsel", [128, 128], F32, isOutput=False)
    out_d = nc.declare_dram_parameter("out", [BPC, 128, NT, N], F32, isOutput=True)

    with tile.TileContext(nc) as tc:
        with (
            tc.tile_pool(name="consts", bufs=1) as consts,
            tc.tile_pool(name="xpool", bufs=1) as xpool,
            tc.tile_pool(name="rxpool", bufs=2) as rxpool,
            tc.tile_pool(name="rbpool", bufs=2) as rbpool,
            tc.tile_pool(name="hnpool", bufs=2) as hnpool,
            tc.tile_pool(name="qkpool", bufs=qk_bufs) as qkpool,
            tc.tile_pool(name="vpool", bufs=2) as vpool,
            tc.tile_pool(name="epool", bufs=2) as epool,
            tc.tile_pool(name="opool", bufs=2) as opool,
            tc.tile_pool(name="dpool", bufs=4) as dpool,
            tc.tile_pool(name="outpool", bufs=out_bufs) as outpool,
            tc.tile_pool(name="spool", bufs=2) as spool,
            tc.tile_pool(name="psum", bufs=3, space="PSUM") as psum,
        ):
            # ---- constants / weights ----
            wqkv_sb = consts.tile([128, NT, 3 * C], BF16)
            nc.sync.dma_start(out=wqkv_sb, in_=wqkv_d[:])
            wp_sb = consts.tile([128, NT, C], BF16)
            nc.sync.dma_start(out=wp_sb, in_=wp_d[:])
            qkvb_sb = consts.tile([128, 2 * NT], F32)
            nc.sync.dma_start(out=qkvb_sb, in_=qkvb_d[:])
            vbias_sb = consts.tile([128, NH * 65], BF16)
            nc.sync.dma_start(out=vbias_sb, in_=vbias_d[:])
            pb_sb = consts.tile([128, NT], F32)
            nc.sync.dma_start(out=pb_sb, in_=pb_d[:])
            nw_sb = consts.tile([128, NT], F32)
            nc.sync.dma_start(out=nw_sb, in_=nw_d[:])
            nb_sb = consts.tile([128, NT], F32)
            nc.sync.dma_start(out=nb_sb, in_=nb_d[:])
            gsel_sb = consts.tile([128, 128], F32)
            nc.sync.dma_start(out=gsel_sb, in_=gsel_d[:])
            eps_sb = consts.tile([128, 1], F32)
            nc.vector.memset(eps_sb, EPS)
            ones64 = consts.tile([1, 64], BF16)
            nc.vector.memset(ones64, 1.0)

            BNS = nc.vector.BN_STATS_DIM   # 6
            BNA = nc.vector.BN_AGGR_DIM    # 2

            # ---- groupnorm for both batches (all sqrt ACT ops before any exp) ----
            state = {}
            for b in range(BPC):
                x_sb = xpool.tile([128, NT, N], F32)
                nc.sync.dma_start(out=x_sb, in_=x_d[b])

                stats4 = spool.tile([128, 2 * NT], F32)
                for t in range(NT):
                    bnstat = spool.tile([128, 2, BNS], F32)
                    xv = x_sb[:, t, :].rearrange("p (s n) -> p s n", s=2)
                    for s in range(2):
                        nc.vector.bn_stats(out=bnstat[:, s, :], in_=xv[:, s, :])
                    mv = spool.tile([128, BNA], F32)
                    nc.vector.bn_aggr(out=mv, in_=bnstat)
                    nc.vector.tensor_copy(out=stats4[:, t : t + 1], in_=mv[:, 0:1])
                    nc.vector.scalar_tensor_tensor(
                        out=stats4[:, NT + t : NT + t + 1],
                        in0=mv[:, 0:1],
                        scalar=mv[:, 0:1],
                        in1=mv[:, 1:2],
                        op0=mybir.AluOpType.mult,
                        op1=mybir.AluOpType.add,
                    )

                pst = psum.tile([128, 2 * NT], F32, tag="s")
                nc.tensor.matmul(pst[:], gsel_sb[:], stats4[:], start=True, stop=True)

                mean4 = spool.tile([128, NT], F32)
                nc.vector.tensor_scalar_mul(out=mean4, in0=pst[:, 0:NT], scalar1=1.0 / GS)
                msq4 = spool.tile([128, NT], F32)
                nc.vector.tensor_mul(out=msq4, in0=mean4, in1=mean4)
                var4 = spool.tile([128, NT], F32)
                nc.vector.scalar_tensor_tensor(
                    out=var4,
                    in0=pst[:, NT : 2 * NT],
                    scalar=1.0 / GS,
                    in1=msq4,
                    op0=mybir.AluOpType.mult,
                    op1=mybir.AluOpType.subtract,
                )
                std4 = spool.tile([128, NT], F32)
                nc.scalar.activation(
                    out=std4, in_=var4, func=mybir.ActivationFunctionType.Sqrt,
                    bias=eps_sb[:, 0:1], scale=1.0,
                )
                rstd4 = spool.tile([128, NT], F32)
                nc.vector.reciprocal(out=rstd4, in_=std4)
                a4 = spool.tile([128, NT], F32)
                nc.vector.tensor_mul(out=a4, in0=rstd4, in1=nw_sb)
                mb4 = spool.tile([128, NT], F32)
                nc.vector.tensor_mul(out=mb4, in0=mean4, in1=a4)
                b4 = spool.tile([128, NT], F32)
                nc.vector.tensor_sub(out=b4, in0=nb_sb, in1=mb4)

                hn = hnpool.tile([128, NT, N], BF16)
                for t in range(NT):
                    nc.vector.tensor_scalar(
                        out=hn[:, t, :],
                        in0=x_sb[:, t, :],
                        scalar1=a4[:, t : t + 1],
                        scalar2=b4[:, t : t + 1],
                        op0=mybir.AluOpType.mult,
                        op1=mybir.AluOpType.add,
                    )
                state[b] = {"x": x_sb, "hn": hn}

            # ---- emission helpers (PE queue is in-order: keep ScalarE fed) ----
            def emit_qk_chunk(st, j):
                """q chunk j and k chunk j of qkT (rows j*128 / (NT+j)*128)."""
                hn = st["hn"]
                for rb in (j, NT + j):
                    ps = psum.tile([128, N], F32, tag="s", name="ps_qk")
                    for half in range(2):
                        for kc in range(NT):
                            nc.tensor.matmul(
                                ps[:, half * 512 : (half + 1) * 512],
                                wqkv_sb[:, kc, rb * 128 : (rb + 1) * 128],
                                hn[:, kc, half * 512 : (half + 1) * 512],
                                start=(kc == 0),
                                stop=(kc == NT - 1),
                            )
                    nc.vector.tensor_scalar_add(
                        out=st["qkT"][:, rb, :], in0=ps[:],
                        scalar1=qkvb_sb[:, rb : rb + 1],
                    )

            def emit_scores_exp(st, j):
                """Concurrent K=64 score matmuls for head pair (2j, 2j+1)."""
                qkT = st["qkT"]
                pair = []
                for i in range(2):
                    h = 2 * j + i
                    poff = (h % 2) * 64
                    qT_h = qkT[poff : poff + 64, h // 2, :]
                    kT_h = qkT[poff : poff + 64, NT + h // 2, :]
                    expT = epool.tile([128, 8, N], BF16, name="expT", tag=f"expT{i}")
                    pair.append((qT_h, kT_h, expT))
                for mb in range(8):
                    pss = [
                        psum.tile([128, N], F32, tag="s", name="ps_s")
                        for _ in range(2)
                    ]
                    for half in range(2):
                        for i in range(2):
                            qT_h, kT_h, _ = pair[i]
                            nc.tensor.matmul(
                                pss[i][:, half * 512 : (half + 1) * 512],
                                kT_h[:, mb * 128 : (mb + 1) * 128],
                                qT_h[:, half * 512 : (half + 1) * 512],
                                start=True,
                                stop=True,
                            )
                    for i in range(2):
                        nc.scalar.activation(
                            out=pair[i][2][:, mb, :], in_=pss[i][:],
                            func=mybir.ActivationFunctionType.Exp, scale=SCALE,
                        )
                return [p[2] for p in pair]

            def emit_av_norm(st, j, expTs):
                v_pad = st["v_pad"]
                o_sb = st["o_sb"]
                for i in range(2):
                    h = 2 * j + i
                    poff = (h % 2) * 64
                    expT = expTs[i]
                    pos = []
                    denbs = []
                    for half in range(2):
                        po = psum.tile([65, 512], F32, tag="o", bufs=2, name="po")
                        for mb in range(8):
                            nc.tensor.matmul(
                                po[:],
                                v_pad[:, mb, h * 65 : (h + 1) * 65],
                                expT[:, mb, half * 512 : (half + 1) * 512],
                                start=(mb == 0),
                                stop=(mb == 7),
                            )
                        denb = dpool.tile([1, 512], BF16, name="denb")
                        with nc.allow_low_precision(reason="softmax denom bf16"):
                            nc.vector.tensor_copy(out=denb, in_=po[64:65, :])
                        pos.append(po)
                        denbs.append(denb)
                    # broadcast denominator over 64 partitions, then one fast
                    # approx reciprocal on the whole (64, N) tile
                    pbc = psum.tile([64, N], F32, tag="s", name="pbc")
                    for half in range(2):
                        nc.tensor.matmul(
                            pbc[:, half * 512 : (half + 1) * 512],
                            ones64[:],
                            denbs[half][:],
                            start=True,
                            stop=True,
                        )
                    recip_bc = rbpool.tile([64, N], F32, name="recip_bc")
                    nc.vector.reciprocal_approx_fast(out=recip_bc, in_=pbc[:])
                    # normalize fused with the PSUM->SBUF evacuation
                    for half in range(2):
                        nc.vector.tensor_mul(
                            out=o_sb[
                                poff : poff + 64, h // 2, half * 512 : (half + 1) * 512
                            ],
                            in0=pos[half][0:64, :],
                            in1=recip_bc[:, half * 512 : (half + 1) * 512],
                        )

            def emit_proj_rb(st, b, rb):
                o_sb = st["o_sb"]
                pp = psum.tile([128, N], F32, tag="s", name="pp")
                for half in range(2):
                    for kc in range(NT):
                        nc.tensor.matmul(
                            pp[:, half * 512 : (half + 1) * 512],
                            wp_sb[:, kc, rb * 128 : (rb + 1) * 128],
                            o_sb[:, kc, half * 512 : (half + 1) * 512],
                            start=(kc == 0),
                            stop=(kc == NT - 1),
                        )
                rx = rxpool.tile([128, N], F32, name="rx")
                nc.sync.dma_start(out=rx, in_=x_d[b, :, rb, :])
                out_sb = outpool.tile([128, N], F32, name="out_sb")
                nc.vector.scalar_tensor_tensor(
                    out=out_sb,
                    in0=pp[:],
                    scalar=pb_sb[:, rb : rb + 1],
                    in1=rx[:],
                    op0=mybir.AluOpType.add,
                    op1=mybir.AluOpType.add,
                )
                nc.sync.dma_start(out=out_d[b, :, rb, :], in_=out_sb[:])

            # ---- main interleaved schedule ----
            # Software pipeline: scores/exp of pair j+1 are emitted BEFORE
            # AV/normalize of pair j, so ScalarE (the bottleneck) streams
            # exp blocks back to back while PE trails with AV + fillers.
            def emit_v_piece(st, mbp):
                hn = st["hn"]
                v_pad = st["v_pad"]
                psv = psum.tile([128, N], F32, tag="s", name="psv")
                for half in range(2):
                    mb = 2 * mbp + half
                    for kc in range(NT):
                        nc.tensor.matmul(
                            psv[:, half * 512 : (half + 1) * 512],
                            hn[:, kc, mb * 128 : (mb + 1) * 128],
                            wqkv_sb[:, kc, 2 * C : 3 * C],
                            start=(kc == 0),
                            stop=(kc == NT - 1),
                        )
                    nc.vector.tensor_tensor(
                        out=v_pad[:, mb, :].rearrange("p (h c) -> p h c", c=65)[
                            :, :, 0:64
                        ],
                        in0=psv[:, half * 512 : (half + 1) * 512].rearrange(
                            "p (h c) -> p h c", c=64
                        ),
                        in1=vbias_sb.rearrange("p (h c) -> p h c", c=65)[:, :, 0:64],
                        op=mybir.AluOpType.add,
                    )

            def setup_batch(b):
                st = state[b]
                st["qkT"] = qkpool.tile([128, 2 * NT, N], BF16, name="qkT")
                st["v_pad"] = vpool.tile([128, 8, NH * 65], BF16, name="v_pad")
                ones_view = st["v_pad"].rearrange("p m (h c) -> p m h c", c=65)[
                    :, :, :, 64:65
                ]
                nc.vector.memset(ones_view, 1.0)
                st["o_sb"] = opool.tile([128, NT, N], BF16, name="o_sb")

            setup_batch(0)
            emit_qk_chunk(state[0], 0)

            FILLERS = {
                (0, 0): [("qk", 0, 1), ("v", 0, 0), ("v", 0, 1), ("v", 0, 2), ("v", 0, 3)],
                (0, 1): [("qk", 0, 2)],
                (0, 2): [("qk", 0, 3)],
                (0, 3): [("setup", 1), ("qk", 1, 0), ("v", 1, 0), ("v", 1, 1)],
                (1, 0): [("qk", 1, 1), ("v", 1, 2), ("v", 1, 3)],
                (1, 1): [("qk", 1, 2), ("proj", 0, 0), ("proj", 0, 1)],
                (1, 2): [("qk", 1, 3), ("proj", 0, 2), ("proj", 0, 3)],
                (1, 3): [],
            }

            pending = None
            for b in range(BPC):
                st = state[b]
                for j in range(4):
                    expTs = emit_scores_exp(st, j)
                    if pending is not None:
                        emit_av_norm(*pending)
                    pending = (st, j, expTs)
                    for f in FILLERS[(b, j)]:
                        if f[0] == "qk":
                            emit_qk_chunk(state[f[1]], f[2])
                        elif f[0] == "v":
                            emit_v_piece(state[f[1]], f[2])
                        elif f[0] == "setup":
                            setup_batch(f[1])
                        elif f[0] == "proj":
                            emit_proj_rb(state[f[1]], f[1], f[2])

            emit_av_norm(*pending)
            for rb in range(NT):
                emit_proj_rb(state[BPC - 1], BPC - 1, rb)

    nc.finalize()
    return nc


_PROGRAM = None


def _get_program():
    global _PROGRAM
    if _PROGRAM is None:
        _PROGRAM = build_program()
    return _PROGRAM


def _prep_inputs(x, norm_w, norm_b, qkv_w, qkv_b, proj_w, proj_b):
    x = np.asarray(x, np.float32)
    xs = np.ascontiguousarray(
        x.reshape(B, NT, 128, N).transpose(0, 2, 1, 3)
    )  # (B, 128, NT, N)

    wqkvT = np.asarray(qkv_w, np.float32).T  # (C, 3C)
    wqkv = np.ascontiguousarray(
        wqkvT.reshape(NT, 128, 3 * C).transpose(1, 0, 2)
    ).astype(ml_dtypes.bfloat16)
    wpT = np.asarray(proj_w, np.float32).T
    wp = np.ascontiguousarray(wpT.reshape(NT, 128, C).transpose(1, 0, 2)).astype(
        ml_dtypes.bfloat16
    )

    qkv_b = np.asarray(qkv_b, np.float32)
    qkvb8 = np.ascontiguousarray(qkv_b[: 2 * C].reshape(2 * NT, 128).T)  # (128, 8)
    vb = np.zeros((NH, 65), np.float32)
    vb[:, :64] = qkv_b[2 * C :].reshape(NH, 64)
    vbias = np.ascontiguousarray(
        np.broadcast_to(vb.reshape(1, NH * 65), (128, NH * 65))
    )
    pb4 = np.ascontiguousarray(np.asarray(proj_b, np.float32).reshape(NT, 128).T)
    nw4 = np.ascontiguousarray(np.asarray(norm_w, np.float32).reshape(NT, 128).T)
    nb4 = np.ascontiguousarray(np.asarray(norm_b, np.float32).reshape(NT, 128).T)

    idx = np.arange(128)
    gsel = (idx[:, None] // GS == idx[None, :] // GS).astype(np.float32)

    shared = {
        "wqkv": wqkv, "wp": wp, "qkvb": qkvb8, "vbias": vbias, "pb": pb4,
        "nw": nw4, "nb": nb4, "gsel": gsel,
    }
    in_maps = [
        {"x": np.ascontiguousarray(xs[c * BPC : (c + 1) * BPC]), **shared}
        for c in range(NCORES)
    ]
    return in_maps


def _assemble(results):
    outs = np.concatenate(
        [results[c]["out"] for c in range(NCORES)], axis=0
    )  # (B, 128, NT, N)
    return np.ascontiguousarray(
        outs.transpose(0, 2, 1, 3).reshape(B, C, HH, WW)
    ).astype(np.float32)


def kernel(x, norm_w, norm_b, qkv_w, qkv_b, proj_w, proj_b, _trace=False):
    from concourse.bass_utils import run_bass_kernel_spmd

    nc = _get_program()
    in_maps = _prep_inputs(x, norm_w, norm_b, qkv_w, qkv_b, proj_w, proj_b)
    res = run_bass_kernel_spmd(nc, in_maps, list(range(NCORES)), trace=_trace)
    out = _assemble(res.results)
    if _trace:
        return out, res
    return out


# revision 28
# speedup vs baseline: 1.2174x; 1.0271x over previous
"""AttentionBlock (GroupNorm + 8-head self-attention + proj + residual) on 8 trn2 cores.

Sharding: data-parallel over batch (16 batches -> 2 per core), no collectives.

Per-core device program (per batch):
  - GroupNorm(32, 512): bn_stats per 128-channel tile -> per-channel [mean, E[x^2]]
    -> cross-partition group reduce via a (128,128) group-indicator fp32 matmul
    -> per-channel scale/bias -> hn (bf16).
  - QKV 1x1 conv as matmuls (bf16): q,k produced in (channel, pixel) layout;
    v produced in (pixel, channel) layout, padded with a ones column per head.
  - Attention head-pair j=(2j, 2j+1): the two heads sit at partition offsets
    0/64 of the same qkT chunk, so their K=64 score matmuls occupy disjoint
    PE row-groups (tile_position auto-derived from base partition) and run
    concurrently. exp on ScalarE straight from PSUM (scores bounded ~6.5 ->
    no max subtraction); ScalarE is the kernel bottleneck, so qkv/v/proj
    matmul work is interleaved between score blocks to keep it fed.
  - AV matmul with the ones column producing the softmax denominator as psum
    row 64 (two 1-bank halves for pipelining). Denominator reciprocal ->
    broadcast over 64 partitions via a K=1 ones matmul -> normalize o.
  - proj matmul + (bias + residual) fused in one DVE op -> DMA out.
"""

import numpy as np
import ml_dtypes

import concourse.bass as bass
import concourse.tile as tile
from concourse import bacc, mybir

B, C, HH, WW = 16, 512, 32, 32
N = HH * WW          # 1024 pixels
NH, HD = 8, 64       # heads, head dim
NG, GS = 32, 16      # groups, channels per group
NCORES = 8
BPC = B // NCORES    # batches per core
NT = C // 128        # channel tiles of 128
EPS = 1e-5
SCALE = HD ** -0.5

F32 = mybir.dt.float32
BF16 = mybir.dt.bfloat16


def build_program(qk_bufs=1, out_bufs=2):
    nc = bacc.Bacc(None, target_bir_lowering=False, debug=False)

    x_d = nc.declare_dram_parameter("x", [BPC, 128, NT, N], F32, isOutput=False)
    wqkv_d = nc.declare_dram_parameter("wqkv", [128, NT, 3 * C], BF16, isOutput=False)
    wp_d = nc.declare_dram_parameter("wp", [128, NT, C], BF16, isOutput=False)
    qkvb_d = nc.declare_dram_parameter("qkvb", [128, 2 * NT], F32, isOutput=False)
    vbias_d = nc.declare_dram_parameter("vbias", [128, NH * 65], BF16, isOutput=False)
    pb_d = nc.declare_dram_parameter("pb", [128, NT], F32, isOutput=False)
    nw_d = nc.declare_dram_parameter("nw", [128, NT], F32, isOutput=False)
    nb_d = nc.declare_dram_parameter("nb", [128, NT], F32, isOutput=False)
    gsel_d = nc.declare_dram_parameter("gsel", [128, 128], F32, isOutput=False)
    out_d = nc.declare_dram_parameter("out", [BPC, 128, NT, N], F32, isOutput=True)

    with tile.TileContext(nc) as tc:
        with (
            tc.tile_pool(name="consts", bufs=1) as consts,
            tc.tile_pool(name="xpool", bufs=1) as xpool,
            tc.tile_pool(name="rxpool", bufs=2) as rxpool,
            tc.tile_pool(name="rbpool", bufs=2) as rbpool,
            tc.tile_pool(name="hnpool", bufs=2) as hnpool,
            tc.tile_pool(name="qkpool", bufs=qk_bufs) as qkpool,
            tc.tile_pool(name="vpool", bufs=2) as vpool,
            tc.tile_pool(name="epool", bufs=2) as epool,
            tc.tile_pool(name="opool", bufs=2) as opool,
            tc.tile_pool(name="dpool", bufs=4) as dpool,
            tc.tile_pool(name="outpool", bufs=out_bufs) as outpool,
            tc.tile_pool(name="spool", bufs=2) as spool,
            tc.tile_pool(name="psum", bufs=2, space="PSUM") as psum,
        ):
            # ---- constants / weights ----
            wqkv_sb = consts.tile([128, NT, 3 * C], BF16)
            nc.sync.dma_start(out=wqkv_sb, in_=wqkv_d[:])
            wp_sb = consts.tile([128, NT, C], BF16)
            nc.sync.dma_start(out=wp_sb, in_=wp_d[:])
            qkvb_sb = consts.tile([128, 2 * NT], F32)
            nc.sync.dma_start(out=qkvb_sb, in_=qkvb_d[:])
            vbias_sb = consts.tile([128, NH * 65], BF16)
            nc.sync.dma_start(out=vbias_sb, in_=vbias_d[:])
            pb_sb = consts.tile([128, NT], F32)
            nc.sync.dma_start(out=pb_sb, in_=pb_d[:])
            nw_sb = consts.tile([128, NT], F32)
            nc.sync.dma_start(out=nw_sb, in_=nw_d[:])
            nb_sb = consts.tile([128, NT], F32)
            nc.sync.dma_start(out=nb_sb, in_=nb_d[:])
            gsel_sb = consts.tile([128, 128], F32)
            nc.sync.dma_start(out=gsel_sb, in_=gsel_d[:])
            eps_sb = consts.tile([128, 1], F32)
            nc.vector.memset(eps_sb, EPS)
            ones64 = consts.tile([1, 64], BF16)
            nc.vector.memset(ones64, 1.0)

            BNS = nc.vector.BN_STATS_DIM   # 6
            BNA = nc.vector.BN_AGGR_DIM    # 2

            # ---- groupnorm for both batches (all sqrt ACT ops before any exp) ----
            state = {}
            for b in range(BPC):
                x_sb = xpool.tile([128, NT, N], F32)
                for t in range(NT):
                    nc.sync.dma_start(out=x_sb[:, t, :], in_=x_d[b, :, t, :])

                stats4 = spool.tile([128, 2 * NT], F32)
                for t in range(NT):
                    bnstat = spool.tile([128, 2, BNS], F32)
                    xv = x_sb[:, t, :].rearrange("p (s n) -> p s n", s=2)
                    for s in range(2):
                        nc.vector.bn_stats(out=bnstat[:, s, :], in_=xv[:, s, :])
                    mv = spool.tile([128, BNA], F32)
                    nc.vector.bn_aggr(out=mv, in_=bnstat)
                    nc.vector.tensor_copy(out=stats4[:, t : t + 1], in_=mv[:, 0:1])
                    nc.vector.scalar_tensor_tensor(
                        out=stats4[:, NT + t : NT + t + 1],
                        in0=mv[:, 0:1],
                        scalar=mv[:, 0:1],
                        in1=mv[:, 1:2],
                        op0=mybir.AluOpType.mult,
                        op1=mybir.AluOpType.add,
                    )

                pst = psum.tile([128, 2 * NT], F32, tag="w", bufs=1)
                nc.tensor.matmul(pst[:], gsel_sb[:], stats4[:], start=True, stop=True)

                mean4 = spool.tile([128, NT], F32)
                nc.vector.tensor_scalar_mul(out=mean4, in0=pst[:, 0:NT], scalar1=1.0 / GS)
                msq4 = spool.tile([128, NT], F32)
                nc.vector.tensor_mul(out=msq4, in0=mean4, in1=mean4)
                var4 = spool.tile([128, NT], F32)
                nc.vector.scalar_tensor_tensor(
                    out=var4,
                    in0=pst[:, NT : 2 * NT],
                    scalar=1.0 / GS,
                    in1=msq4,
                    op0=mybir.AluOpType.mult,
                    op1=mybir.AluOpType.subtract,
                )
                std4 = spool.tile([128, NT], F32)
                nc.scalar.activation(
                    out=std4, in_=var4, func=mybir.ActivationFunctionType.Sqrt,
                    bias=eps_sb[:, 0:1], scale=1.0,
                )
                rstd4 = spool.tile([128, NT], F32)
                nc.vector.reciprocal(out=rstd4, in_=std4)
                a4 = spool.tile([128, NT], F32)
                nc.vector.tensor_mul(out=a4, in0=rstd4, in1=nw_sb)
                mb4 = spool.tile([128, NT], F32)
                nc.vector.tensor_mul(out=mb4, in0=mean4, in1=a4)
                b4 = spool.tile([128, NT], F32)
                nc.vector.tensor_sub(out=b4, in0=nb_sb, in1=mb4)

                hn = hnpool.tile([128, NT, N], BF16)
                for t in range(NT):
                    nc.vector.tensor_scalar(
                        out=hn[:, t, :],
                        in0=x_sb[:, t, :],
                        scalar1=a4[:, t : t + 1],
                        scalar2=b4[:, t : t + 1],
                        op0=mybir.AluOpType.mult,
                        op1=mybir.AluOpType.add,
                    )
                state[b] = {"x": x_sb, "hn": hn}

            # ---- emission helpers (PE queue is in-order: keep ScalarE fed) ----
            def emit_qk_chunk(st, j):
                """q chunk j and k chunk j of qkT (rows j*128 / (NT+j)*128)."""
                hn = st["hn"]
                for rb in (j, NT + j):
                    ps = psum.tile([128, N], F32, tag="w", bufs=1, name="ps_qk")
                    for half in range(2):
                        for kc in range(NT):
                            nc.tensor.matmul(
                                ps[:, half * 512 : (half + 1) * 512],
                                wqkv_sb[:, kc, rb * 128 : (rb + 1) * 128],
                                hn[:, kc, half * 512 : (half + 1) * 512],
                                start=(kc == 0),
                                stop=(kc == NT - 1),
                            )
                    nc.vector.tensor_scalar_add(
                        out=st["qkT"][:, rb, :], in0=ps[:],
                        scalar1=qkvb_sb[:, rb : rb + 1],
                    )

            def emit_scores_exp(st, j):
                """Concurrent K=64 score matmuls for head pair (2j, 2j+1)."""
                qkT = st["qkT"]
                pair = []
                for i in range(2):
                    h = 2 * j + i
                    poff = (h % 2) * 64
                    qT_h = qkT[poff : poff + 64, h // 2, :]
                    kT_h = qkT[poff : poff + 64, NT + h // 2, :]
                    expT = epool.tile([128, 8, N], BF16, name="expT", tag=f"expT{i}")
                    pair.append((qT_h, kT_h, expT))
                for mb in range(8):
                    pss = [
                        psum.tile([128, N], F32, tag="sc", bufs=2, name="ps_s")
                        for _ in range(2)
                    ]
                    for half in range(2):
                        for i in range(2):
                            qT_h, kT_h, _ = pair[i]
                            nc.tensor.matmul(
                                pss[i][:, half * 512 : (half + 1) * 512],
                                kT_h[:, mb * 128 : (mb + 1) * 128],
                                qT_h[:, half * 512 : (half + 1) * 512],
                                start=True,
                                stop=True,
                            )
                    for i in range(2):
                        nc.scalar.activation(
                            out=pair[i][2][:, mb, :], in_=pss[i][:],
                            func=mybir.ActivationFunctionType.Exp, scale=SCALE,
                        )
                return [p[2] for p in pair]

            def emit_av_norm(st, j, expTs):
                v_pad = st["v_pad"]
                o_sb = st["o_sb"]
                for i in range(2):
                    h = 2 * j + i
                    poff = (h % 2) * 64
                    expT = expTs[i]
                    pos = []
                    denbs = []
                    for half in range(2):
                        po = psum.tile([65, 512], F32, tag="o", bufs=2, name="po")
                        for mb in range(8):
                            nc.tensor.matmul(
                                po[:],
                                v_pad[:, mb, h * 65 : (h + 1) * 65],
                                expT[:, mb, half * 512 : (half + 1) * 512],
                                start=(mb == 0),
                                stop=(mb == 7),
                            )
                        denb = dpool.tile([1, 512], BF16, name="denb")
                        with nc.allow_low_precision(reason="softmax denom bf16"):
                            nc.vector.tensor_copy(out=denb, in_=po[64:65, :])
                        pos.append(po)
                        denbs.append(denb)
                    # broadcast denominator over 64 partitions, then one fast
                    # approx reciprocal on the whole (64, N) tile
                    pbc = psum.tile([64, N], F32, tag="w", bufs=1, name="pbc")
                    for half in range(2):
                        nc.tensor.matmul(
                            pbc[:, half * 512 : (half + 1) * 512],
                            ones64[:],
                            denbs[half][:],
                            start=True,
                            stop=True,
                        )
                    recip_bc = rbpool.tile([64, N], F32, name="recip_bc")
                    nc.vector.reciprocal_approx_fast(out=recip_bc, in_=pbc[:])
                    # normalize fused with the PSUM->SBUF evacuation
                    for half in range(2):
                        nc.vector.tensor_mul(
                            out=o_sb[
                                poff : poff + 64, h // 2, half * 512 : (half + 1) * 512
                            ],
                            in0=pos[half][0:64, :],
                            in1=recip_bc[:, half * 512 : (half + 1) * 512],
                        )

            def emit_proj_rb(st, b, rb):
                o_sb = st["o_sb"]
                pp = psum.tile([128, N], F32, tag="w", bufs=1, name="pp")
                for half in range(2):
                    for kc in range(NT):
                        nc.tensor.matmul(
                            pp[:, half * 512 : (half + 1) * 512],
                            wp_sb[:, kc, rb * 128 : (rb + 1) * 128],
                            o_sb[:, kc, half * 512 : (half + 1) * 512],
                            start=(kc == 0),
                            stop=(kc == NT - 1),
                        )
                rx = rxpool.tile([128, N], F32, name="rx")
                nc.sync.dma_start(out=rx, in_=x_d[b, :, rb, :])
                out_sb = outpool.tile([128, N], F32, name="out_sb")
                nc.vector.scalar_tensor_tensor(
                    out=out_sb,
                    in0=pp[:],
                    scalar=pb_sb[:, rb : rb + 1],
                    in1=rx[:],
                    op0=mybir.AluOpType.add,
                    op1=mybir.AluOpType.add,
                )
                nc.sync.dma_start(out=out_d[b, :, rb, :], in_=out_sb[:])

            # ---- main interleaved schedule ----
            # Software pipeline: scores/exp of pair j+1 are emitted BEFORE
            # AV/normalize of pair j, so ScalarE (the bottleneck) streams
            # exp blocks back to back while PE trails with AV + fillers.
            def emit_v_piece(st, mbp):
                hn = st["hn"]
                v_pad = st["v_pad"]
                psv = psum.tile([128, N], F32, tag="w", bufs=1, name="psv")
                for half in range(2):
                    mb = 2 * mbp + half
                    for kc in range(NT):
                        nc.tensor.matmul(
                            psv[:, half * 512 : (half + 1) * 512],
                            hn[:, kc, mb * 128 : (mb + 1) * 128],
                            wqkv_sb[:, kc, 2 * C : 3 * C],
                            start=(kc == 0),
                            stop=(kc == NT - 1),
                        )
                    nc.vector.tensor_tensor(
                        out=v_pad[:, mb, :].rearrange("p (h c) -> p h c", c=65)[
                            :, :, 0:64
                        ],
                        in0=psv[:, half * 512 : (half + 1) * 512].rearrange(
                            "p (h c) -> p h c", c=64
                        ),
                        in1=vbias_sb.rearrange("p (h c) -> p h c", c=65)[:, :, 0:64],
                        op=mybir.AluOpType.add,
                    )

            def setup_batch(b):
                st = state[b]
                st["qkT"] = qkpool.tile([128, 2 * NT, N], BF16, name="qkT")
                st["v_pad"] = vpool.tile([128, 8, NH * 65], BF16, name="v_pad")
                ones_view = st["v_pad"].rearrange("p m (h c) -> p m h c", c=65)[
                    :, :, :, 64:65
                ]
                nc.vector.memset(ones_view, 1.0)
                st["o_sb"] = opool.tile([128, NT, N], BF16, name="o_sb")

            setup_batch(0)
            emit_qk_chunk(state[0], 0)

            FILLERS = {
                (0, 0): [("qk", 0, 1), ("v", 0, 0), ("v", 0, 1), ("v", 0, 2), ("v", 0, 3)],
                (0, 1): [("qk", 0, 2)],
                (0, 2): [("qk", 0, 3)],
                (0, 3): [("setup", 1), ("qk", 1, 0), ("v", 1, 0), ("v", 1, 1)],
                (1, 0): [("qk", 1, 1), ("v", 1, 2), ("v", 1, 3)],
                (1, 1): [("qk", 1, 2), ("proj", 0, 0), ("proj", 0, 1)],
                (1, 2): [("qk", 1, 3), ("proj", 0, 2), ("proj", 0, 3)],
                (1, 3): [],
            }

            pending = None
            for b in range(BPC):
                st = state[b]
                for j in range(4):
                    expTs = emit_scores_exp(st, j)
                    if pending is not None:
                        emit_av_norm(*pending)
                    pending = (st, j, expTs)
                    for f in FILLERS[(b, j)]:
                        if f[0] == "qk":
                            emit_qk_chunk(state[f[1]], f[2])
                        elif f[0] == "v":
                            emit_v_piece(state[f[1]], f[2])
                        elif f[0] == "setup":
                            setup_batch(f[1])
                        elif f[0] == "proj":
                            emit_proj_rb(state[f[1]], f[1], f[2])

            emit_av_norm(*pending)
            for rb in range(NT):
                emit_proj_rb(state[BPC - 1], BPC - 1, rb)

    nc.finalize()
    return nc


_PROGRAM = None


def _get_program():
    global _PROGRAM
    if _PROGRAM is None:
        _PROGRAM = build_program()
    return _PROGRAM


def _prep_inputs(x, norm_w, norm_b, qkv_w, qkv_b, proj_w, proj_b):
    x = np.asarray(x, np.float32)
    xs = np.ascontiguousarray(
        x.reshape(B, NT, 128, N).transpose(0, 2, 1, 3)
    )  # (B, 128, NT, N)

    wqkvT = np.asarray(qkv_w, np.float32).T  # (C, 3C)
    wqkv = np.ascontiguousarray(
        wqkvT.reshape(NT, 128, 3 * C).transpose(1, 0, 2)
    ).astype(ml_dtypes.bfloat16)
    wpT = np.asarray(proj_w, np.float32).T
    wp = np.ascontiguousarray(wpT.reshape(NT, 128, C).transpose(1, 0, 2)).astype(
        ml_dtypes.bfloat16
    )

    qkv_b = np.asarray(qkv_b, np.float32)
    qkvb8 = np.ascontiguousarray(qkv_b[: 2 * C].reshape(2 * NT, 128).T)  # (128, 8)
    vb = np.zeros((NH, 65), np.float32)
    vb[:, :64] = qkv_b[2 * C :].reshape(NH, 64)
    vbias = np.ascontiguousarray(
        np.broadcast_to(vb.reshape(1, NH * 65), (128, NH * 65))
    )
    pb4 = np.ascontiguousarray(np.asarray(proj_b, np.float32).reshape(NT, 128).T)
    nw4 = np.ascontiguousarray(np.asarray(norm_w, np.float32).reshape(NT, 128).T)
    nb4 = np.ascontiguousarray(np.asarray(norm_b, np.float32).reshape(NT, 128).T)

    idx = np.arange(128)
    gsel = (idx[:, None] // GS == idx[None, :] // GS).astype(np.float32)

    shared = {
        "wqkv": wqkv, "wp": wp, "qkvb": qkvb8, "vbias": vbias, "pb": pb4,
        "nw": nw4, "nb": nb4, "gsel": gsel,
    }
    in_maps = [
        {"x": np.ascontiguousarray(xs[c * BPC : (c + 1) * BPC]), **shared}
        for c in range(NCORES)
    ]
    return in_maps


def _assemble(results):
    outs = np.concatenate(
        [results[c]["out"] for c in range(NCORES)], axis=0
    )  # (B, 128, NT, N)
    return np.ascontiguousarray(
        outs.transpose(0, 2, 1, 3).reshape(B, C, HH, WW)
    ).astype(np.float32)


def kernel(x, norm_w, norm_b, qkv_w, qkv_b, proj_w, proj_b, _trace=False):
    from concourse.bass_utils import run_bass_kernel_spmd

    nc = _get_program()
    in_maps = _prep_inputs(x, norm_w, norm_b, qkv_w, qkv_b, proj_w, proj_b)
    res = run_bass_kernel_spmd(nc, in_maps, list(range(NCORES)), trace=_trace)
    out = _assemble(res.results)
    if _trace:
        return out, res
    return out


# revision 30
# speedup vs baseline: 1.3341x; 1.0958x over previous
"""AttentionBlock (GroupNorm + 8-head self-attention + proj + residual) on 8 trn2 cores.

Sharding: data-parallel over batch (16 batches -> 2 per core), no collectives.

Per-core device program (per batch):
  - GroupNorm(32, 512): bn_stats per 128-channel tile -> per-channel [mean, E[x^2]]
    -> cross-partition group reduce via a (128,128) group-indicator fp32 matmul
    -> per-channel scale/bias -> hn (bf16).
  - QKV 1x1 conv as matmuls (bf16): q,k produced in (channel, pixel) layout;
    v produced in (pixel, channel) layout, padded with a ones column per head.
  - Attention head-pair j=(2j, 2j+1): the two heads sit at partition offsets
    0/64 of the same qkT chunk, so their K=64 score matmuls occupy disjoint
    PE row-groups (tile_position auto-derived from base partition) and run
    concurrently. exp on ScalarE straight from PSUM (scores bounded ~6.5 ->
    no max subtraction); ScalarE is the kernel bottleneck, so qkv/v/proj
    matmul work is interleaved between score blocks to keep it fed.
  - AV matmul with the ones column producing the softmax denominator as psum
    row 64 (two 1-bank halves for pipelining). Denominator reciprocal ->
    broadcast over 64 partitions via a K=1 ones matmul -> normalize o.
  - proj matmul + (bias + residual) fused in one DVE op -> DMA out.
"""

import numpy as np
import ml_dtypes

import concourse.bass as bass
import concourse.tile as tile
from concourse import bacc, mybir

B, C, HH, WW = 16, 512, 32, 32
N = HH * WW          # 1024 pixels
NH, HD = 8, 64       # heads, head dim
NG, GS = 32, 16      # groups, channels per group
NCORES = 8
BPC = B // NCORES    # batches per core
NT = C // 128        # channel tiles of 128
EPS = 1e-5
SCALE = HD ** -0.5

F32 = mybir.dt.float32
BF16 = mybir.dt.bfloat16


def build_program(qk_bufs=1, out_bufs=2):
    nc = bacc.Bacc(None, target_bir_lowering=False, debug=False)

    x_d = nc.declare_dram_parameter("x", [BPC, 128, NT, N], F32, isOutput=False)
    wqkv_d = nc.declare_dram_parameter("wqkv", [128, NT, 3 * C], BF16, isOutput=False)
    wp_d = nc.declare_dram_parameter("wp", [128, NT, C], BF16, isOutput=False)
    qkvb_d = nc.declare_dram_parameter("qkvb", [128, 2 * NT], F32, isOutput=False)
    vbias_d = nc.declare_dram_parameter("vbias", [128, NH * 65], BF16, isOutput=False)
    pb_d = nc.declare_dram_parameter("pb", [128, NT], F32, isOutput=False)
    nw_d = nc.declare_dram_parameter("nw", [128, NT], F32, isOutput=False)
    nb_d = nc.declare_dram_parameter("nb", [128, NT], F32, isOutput=False)
    gsel_d = nc.declare_dram_parameter("gsel", [128, 128], F32, isOutput=False)
    out_d = nc.declare_dram_parameter("out", [BPC, 128, NT, N], F32, isOutput=True)

    with tile.TileContext(nc) as tc:
        with (
            tc.tile_pool(name="consts", bufs=1) as consts,
            tc.tile_pool(name="xpool", bufs=1) as xpool,
            tc.tile_pool(name="rxpool", bufs=2) as rxpool,
            tc.tile_pool(name="rbpool", bufs=2) as rbpool,
            tc.tile_pool(name="hnpool", bufs=2) as hnpool,
            tc.tile_pool(name="qkpool", bufs=qk_bufs) as qkpool,
            tc.tile_pool(name="vpool", bufs=2) as vpool,
            tc.tile_pool(name="epool", bufs=2) as epool,
            tc.tile_pool(name="opool", bufs=2) as opool,
            tc.tile_pool(name="dpool", bufs=4) as dpool,
            tc.tile_pool(name="outpool", bufs=out_bufs) as outpool,
            tc.tile_pool(name="spool", bufs=2) as spool,
            tc.tile_pool(name="psum", bufs=2, space="PSUM") as psum,
        ):
            # ---- constants / weights ----
            wqkv_sb = consts.tile([128, NT, 3 * C], BF16)
            nc.sync.dma_start(out=wqkv_sb, in_=wqkv_d[:])
            wp_sb = consts.tile([128, NT, C], BF16)
            nc.sync.dma_start(out=wp_sb, in_=wp_d[:])
            qkvb_sb = consts.tile([128, 2 * NT], F32)
            nc.sync.dma_start(out=qkvb_sb, in_=qkvb_d[:])
            vbias_sb = consts.tile([128, NH * 65], BF16)
            nc.sync.dma_start(out=vbias_sb, in_=vbias_d[:])
            pb_sb = consts.tile([128, NT], F32)
            nc.sync.dma_start(out=pb_sb, in_=pb_d[:])
            nw_sb = consts.tile([128, NT], F32)
            nc.sync.dma_start(out=nw_sb, in_=nw_d[:])
            nb_sb = consts.tile([128, NT], F32)
            nc.sync.dma_start(out=nb_sb, in_=nb_d[:])
            gsel_sb = consts.tile([128, 128], F32)
            nc.sync.dma_start(out=gsel_sb, in_=gsel_d[:])
            eps_sb = consts.tile([128, 1], F32)
            nc.vector.memset(eps_sb, EPS)
            ones64 = consts.tile([1, 64], BF16)
            nc.vector.memset(ones64, 1.0)

            BNS = nc.vector.BN_STATS_DIM   # 6
            BNA = nc.vector.BN_AGGR_DIM    # 2

            # ---- groupnorm for both batches (all sqrt ACT ops before any exp) ----
            state = {}
            for b in range(BPC):
                x_sb = xpool.tile([128, NT, N], F32)
                for t in range(NT):
                    nc.sync.dma_start(out=x_sb[:, t, :], in_=x_d[b, :, t, :])

                stats4 = spool.tile([128, 2 * NT], F32)
                for t in range(NT):
                    bnstat = spool.tile([128, 2, BNS], F32)
                    xv = x_sb[:, t, :].rearrange("p (s n) -> p s n", s=2)
                    for s in range(2):
                        nc.vector.bn_stats(out=bnstat[:, s, :], in_=xv[:, s, :])
                    mv = spool.tile([128, BNA], F32)
                    nc.vector.bn_aggr(out=mv, in_=bnstat)
                    nc.vector.tensor_copy(out=stats4[:, t : t + 1], in_=mv[:, 0:1])
                    nc.vector.scalar_tensor_tensor(
                        out=stats4[:, NT + t : NT + t + 1],
                        in0=mv[:, 0:1],
                        scalar=mv[:, 0:1],
                        in1=mv[:, 1:2],
                        op0=mybir.AluOpType.mult,
                        op1=mybir.AluOpType.add,
                    )

                pst = psum.tile([128, 2 * NT], F32, tag="w", bufs=1)
                nc.tensor.matmul(pst[:], gsel_sb[:], stats4[:], start=True, stop=True)

                mean4 = spool.tile([128, NT], F32)
                nc.vector.tensor_scalar_mul(out=mean4, in0=pst[:, 0:NT], scalar1=1.0 / GS)
                msq4 = spool.tile([128, NT], F32)
                nc.vector.tensor_mul(out=msq4, in0=mean4, in1=mean4)
                var4 = spool.tile([128, NT], F32)
                nc.vector.scalar_tensor_tensor(
                    out=var4,
                    in0=pst[:, NT : 2 * NT],
                    scalar=1.0 / GS,
                    in1=msq4,
                    op0=mybir.AluOpType.mult,
                    op1=mybir.AluOpType.subtract,
                )
                std4 = spool.tile([128, NT], F32)
                nc.scalar.activation(
                    out=std4, in_=var4, func=mybir.ActivationFunctionType.Sqrt,
                    bias=eps_sb[:, 0:1], scale=1.0,
                )
                rstd4 = spool.tile([128, NT], F32)
                nc.vector.reciprocal(out=rstd4, in_=std4)
                a4 = spool.tile([128, NT], F32)
                nc.vector.tensor_mul(out=a4, in0=rstd4, in1=nw_sb)
                mb4 = spool.tile([128, NT], F32)
                nc.vector.tensor_mul(out=mb4, in0=mean4, in1=a4)
                b4 = spool.tile([128, NT], F32)
                nc.vector.tensor_sub(out=b4, in0=nb_sb, in1=mb4)

                hn = hnpool.tile([128, NT, N], BF16)
                for t in range(NT):
                    nc.vector.tensor_scalar(
                        out=hn[:, t, :],
                        in0=x_sb[:, t, :],
                        scalar1=a4[:, t : t + 1],
                        scalar2=b4[:, t : t + 1],
                        op0=mybir.AluOpType.mult,
                        op1=mybir.AluOpType.add,
                    )
                state[b] = {"x": x_sb, "hn": hn}

            # ---- emission helpers (PE queue is in-order: keep ScalarE fed) ----
            def emit_qk_chunk(st, j):
                """q chunk j and k chunk j of qkT (rows j*128 / (NT+j)*128)."""
                hn = st["hn"]
                for rb in (j, NT + j):
                    ps = psum.tile([128, N], F32, tag="w", bufs=1, name="ps_qk")
                    for half in range(2):
                        for kc in range(NT):
                            nc.tensor.matmul(
                                ps[:, half * 512 : (half + 1) * 512],
                                wqkv_sb[:, kc, rb * 128 : (rb + 1) * 128],
                                hn[:, kc, half * 512 : (half + 1) * 512],
                                start=(kc == 0),
                                stop=(kc == NT - 1),
                            )
                    nc.vector.tensor_scalar_add(
                        out=st["qkT"][:, rb, :], in0=ps[:],
                        scalar1=qkvb_sb[:, rb : rb + 1],
                    )

            def emit_proj_rb(st, b, rb):
                o_sb = st["o_sb"]
                pp = psum.tile([128, N], F32, tag="w", bufs=1, name="pp")
                for half in range(2):
                    for kc in range(NT):
                        nc.tensor.matmul(
                            pp[:, half * 512 : (half + 1) * 512],
                            wp_sb[:, kc, rb * 128 : (rb + 1) * 128],
                            o_sb[:, kc, half * 512 : (half + 1) * 512],
                            start=(kc == 0),
                            stop=(kc == NT - 1),
                        )
                rx = rxpool.tile([128, N], F32, name="rx")
                nc.sync.dma_start(out=rx, in_=x_d[b, :, rb, :])
                out_sb = outpool.tile([128, N], F32, name="out_sb")
                nc.vector.scalar_tensor_tensor(
                    out=out_sb,
                    in0=pp[:],
                    scalar=pb_sb[:, rb : rb + 1],
                    in1=rx[:],
                    op0=mybir.AluOpType.add,
                    op1=mybir.AluOpType.add,
                )
                nc.sync.dma_start(out=out_d[b, :, rb, :], in_=out_sb[:])

            def emit_scores_unit(st, j, mb, pair):
                """2 concurrent K=64 score matmuls + 2 exps for head pair."""
                pss = [
                    psum.tile([128, N], F32, tag="sc", bufs=2, name="ps_s")
                    for _ in range(2)
                ]
                for half in range(2):
                    for i in range(2):
                        qT_h, kT_h, _ = pair[i]
                        nc.tensor.matmul(
                            pss[i][:, half * 512 : (half + 1) * 512],
                            kT_h[:, mb * 128 : (mb + 1) * 128],
                            qT_h[:, half * 512 : (half + 1) * 512],
                            start=True,
                            stop=True,
                        )
                for i in range(2):
                    nc.scalar.activation(
                        out=pair[i][2][:, mb, :], in_=pss[i][:],
                        func=mybir.ActivationFunctionType.Exp, scale=SCALE,
                    )

            def make_pair(st, j):
                qkT = st["qkT"]
                pair = []
                for i in range(2):
                    h = 2 * j + i
                    poff = (h % 2) * 64
                    qT_h = qkT[poff : poff + 64, h // 2, :]
                    kT_h = qkT[poff : poff + 64, NT + h // 2, :]
                    expT = epool.tile([128, 8, N], BF16, name="expT", tag=f"expT{i}")
                    pair.append((qT_h, kT_h, expT))
                return pair

            def av_units(st, j, pair):
                """AV + normalize for pair j as a list of small PE/DVE units."""
                v_pad = st["v_pad"]
                o_sb = st["o_sb"]
                units = []
                for i in range(2):
                    h = 2 * j + i
                    poff = (h % 2) * 64
                    expT = pair[i][2]
                    box = {}

                    def chain(h=h, expT=expT, half=0, box=box):
                        po = psum.tile([65, 512], F32, tag="o", bufs=2, name="po")
                        for mb in range(8):
                            nc.tensor.matmul(
                                po[:],
                                v_pad[:, mb, h * 65 : (h + 1) * 65],
                                expT[:, mb, half * 512 : (half + 1) * 512],
                                start=(mb == 0),
                                stop=(mb == 7),
                            )
                        denb = dpool.tile([1, 512], BF16, name="denb")
                        with nc.allow_low_precision(reason="softmax denom bf16"):
                            nc.vector.tensor_copy(out=denb, in_=po[64:65, :])
                        box[("po", half)] = po
                        box[("denb", half)] = denb

                    def finish(h=h, poff=poff, box=box):
                        pbc = psum.tile([64, N], F32, tag="w", bufs=1, name="pbc")
                        for half in range(2):
                            nc.tensor.matmul(
                                pbc[:, half * 512 : (half + 1) * 512],
                                ones64[:],
                                box[("denb", half)][:],
                                start=True,
                                stop=True,
                            )
                        recip_bc = rbpool.tile([64, N], F32, name="recip_bc")
                        nc.vector.reciprocal_approx_fast(out=recip_bc, in_=pbc[:])
                        for half in range(2):
                            nc.vector.tensor_mul(
                                out=o_sb[
                                    poff : poff + 64, h // 2,
                                    half * 512 : (half + 1) * 512,
                                ],
                                in0=box[("po", half)][0:64, :],
                                in1=recip_bc[:, half * 512 : (half + 1) * 512],
                            )

                    units.append(lambda c=chain: c(half=0))
                    units.append(lambda c=chain: c(half=1))
                    units.append(finish)
                return units

            def qk_units(st, j):
                us = []
                for rb in (j, NT + j):
                    def u(st=st, rb=rb):
                        hn = st["hn"]
                        ps = psum.tile([128, N], F32, tag="w", bufs=1, name="ps_qk")
                        for half in range(2):
                            for kc in range(NT):
                                nc.tensor.matmul(
                                    ps[:, half * 512 : (half + 1) * 512],
                                    wqkv_sb[:, kc, rb * 128 : (rb + 1) * 128],
                                    hn[:, kc, half * 512 : (half + 1) * 512],
                                    start=(kc == 0),
                                    stop=(kc == NT - 1),
                                )
                        nc.vector.tensor_scalar_add(
                            out=st["qkT"][:, rb, :], in0=ps[:],
                            scalar1=qkvb_sb[:, rb : rb + 1],
                        )
                    us.append(u)
                return us

            def v_unit(st, mbp):
                def u(st=st, mbp=mbp):
                    hn = st["hn"]
                    v_pad = st["v_pad"]
                    psv = psum.tile([128, N], F32, tag="w", bufs=1, name="psv")
                    for half in range(2):
                        mb = 2 * mbp + half
                        for kc in range(NT):
                            nc.tensor.matmul(
                                psv[:, half * 512 : (half + 1) * 512],
                                hn[:, kc, mb * 128 : (mb + 1) * 128],
                                wqkv_sb[:, kc, 2 * C : 3 * C],
                                start=(kc == 0),
                                stop=(kc == NT - 1),
                            )
                        nc.vector.tensor_tensor(
                            out=v_pad[:, mb, :].rearrange("p (h c) -> p h c", c=65)[
                                :, :, 0:64
                            ],
                            in0=psv[:, half * 512 : (half + 1) * 512].rearrange(
                                "p (h c) -> p h c", c=64
                            ),
                            in1=vbias_sb.rearrange("p (h c) -> p h c", c=65)[
                                :, :, 0:64
                            ],
                            op=mybir.AluOpType.add,
                        )
                return u

            def proj_unit(st, b, rb):
                def u():
                    emit_proj_rb(st, b, rb)
                return u

            def setup_batch(b):
                st = state[b]
                st["qkT"] = qkpool.tile([128, 2 * NT, N], BF16, name="qkT")
                st["v_pad"] = vpool.tile([128, 8, NH * 65], BF16, name="v_pad")
                ones_view = st["v_pad"].rearrange("p m (h c) -> p m h c", c=65)[
                    :, :, :, 64:65
                ]
                nc.vector.memset(ones_view, 1.0)
                st["o_sb"] = opool.tile([128, NT, N], BF16, name="o_sb")

            setup_batch(0)
            for u in qk_units(state[0], 0):
                u()

            def fillers(b, j):
                s0, s1 = state[0], state.get(1)
                table = {
                    (0, 0): [v_unit(s0, 0), v_unit(s0, 1), v_unit(s0, 2),
                             v_unit(s0, 3)] + qk_units(s0, 1),
                    (0, 1): qk_units(s0, 2),
                    (0, 2): qk_units(s0, 3),
                    (0, 3): qk_units(s1, 0) + [v_unit(s1, 0), v_unit(s1, 1)],
                    (1, 0): qk_units(s1, 1) + [v_unit(s1, 2), v_unit(s1, 3)],
                    (1, 1): qk_units(s1, 2) + [proj_unit(s0, 0, 0),
                                               proj_unit(s0, 0, 1)],
                    (1, 2): qk_units(s1, 3) + [proj_unit(s0, 0, 2),
                                               proj_unit(s0, 0, 3)],
                    (1, 3): [],
                }
                return table[(b, j)]

            pending = None
            for b in range(BPC):
                st = state[b]
                for j in range(4):
                    if (b, j) == (0, 3):
                        setup_batch(1)
                    pair = make_pair(st, j)
                    units = []
                    if pending is not None:
                        units += av_units(*pending)
                    units += fillers(b, j)
                    pending = (st, j, pair)
                    k = 0
                    for mb in range(8):
                        emit_scores_unit(st, j, mb, pair)
                        target = (mb + 1) * len(units) // 8
                        while k < target:
                            units[k]()
                            k += 1

            for u in av_units(*pending):
                u()
            for rb in range(NT):
                emit_proj_rb(state[BPC - 1], BPC - 1, rb)

    nc.finalize()
    return nc


_PROGRAM = None


def _get_program():
    global _PROGRAM
    if _PROGRAM is None:
        _PROGRAM = build_program()
    return _PROGRAM


def _prep_inputs(x, norm_w, norm_b, qkv_w, qkv_b, proj_w, proj_b):
    x = np.asarray(x, np.float32)
    xs = np.ascontiguousarray(
        x.reshape(B, NT, 128, N).transpose(0, 2, 1, 3)
    )  # (B, 128, NT, N)

    wqkvT = np.asarray(qkv_w, np.float32).T  # (C, 3C)
    wqkv = np.ascontiguousarray(
        wqkvT.reshape(NT, 128, 3 * C).transpose(1, 0, 2)
    ).astype(ml_dtypes.bfloat16)
    wpT = np.asarray(proj_w, np.float32).T
    wp = np.ascontiguousarray(wpT.reshape(NT, 128, C).transpose(1, 0, 2)).astype(
        ml_dtypes.bfloat16
    )

    qkv_b = np.asarray(qkv_b, np.float32)
    qkvb8 = np.ascontiguousarray(qkv_b[: 2 * C].reshape(2 * NT, 128).T)  # (128, 8)
    vb = np.zeros((NH, 65), np.float32)
    vb[:, :64] = qkv_b[2 * C :].reshape(NH, 64)
    vbias = np.ascontiguousarray(
        np.broadcast_to(vb.reshape(1, NH * 65), (128, NH * 65))
    )
    pb4 = np.ascontiguousarray(np.asarray(proj_b, np.float32).reshape(NT, 128).T)
    nw4 = np.ascontiguousarray(np.asarray(norm_w, np.float32).reshape(NT, 128).T)
    nb4 = np.ascontiguousarray(np.asarray(norm_b, np.float32).reshape(NT, 128).T)

    idx = np.arange(128)
    gsel = (idx[:, None] // GS == idx[None, :] // GS).astype(np.float32)

    shared = {
        "wqkv": wqkv, "wp": wp, "qkvb": qkvb8, "vbias": vbias, "pb": pb4,
        "nw": nw4, "nb": nb4, "gsel": gsel,
    }
    in_maps = [
        {"x": np.ascontiguousarray(xs[c * BPC : (c + 1) * BPC]), **shared}
        for c in range(NCORES)
    ]
    return in_maps


def _assemble(results):
    outs = np.concatenate(
        [results[c]["out"] for c in range(NCORES)], axis=0
    )  # (B, 128, NT, N)
    return np.ascontiguousarray(
        outs.transpose(0, 2, 1, 3).reshape(B, C, HH, WW)
    ).astype(np.float32)


def kernel(x, norm_w, norm_b, qkv_w, qkv_b, proj_w, proj_b, _trace=False):
    from concourse.bass_utils import run_bass_kernel_spmd

    nc = _get_program()
    in_maps = _prep_inputs(x, norm_w, norm_b, qkv_w, qkv_b, proj_w, proj_b)
    res = run_bass_kernel_spmd(nc, in_maps, list(range(NCORES)), trace=_trace)
    out = _assemble(res.results)
    if _trace:
        return out, res
    return out


# revision 31
# speedup vs baseline: 1.3783x; 1.0332x over previous
"""AttentionBlock (GroupNorm + 8-head self-attention + proj + residual) on 8 trn2 cores.

Sharding: data-parallel over batch (16 batches -> 2 per core), no collectives.

Per-core device program (per batch):
  - GroupNorm(32, 512): bn_stats per 128-channel tile -> per-channel [mean, E[x^2]]
    -> cross-partition group reduce via a (128,128) group-indicator fp32 matmul
    -> per-channel scale/bias -> hn (bf16).
  - QKV 1x1 conv as matmuls (bf16): q,k produced in (channel, pixel) layout;
    v produced in (pixel, channel) layout, padded with a ones column per head.
  - Attention head-pair j=(2j, 2j+1): the two heads sit at partition offsets
    0/64 of the same qkT chunk, so their K=64 score matmuls occupy disjoint
    PE row-groups (tile_position auto-derived from base partition) and run
    concurrently. exp on ScalarE straight from PSUM (scores bounded ~6.5 ->
    no max subtraction); ScalarE is the kernel bottleneck, so qkv/v/proj
    matmul work is interleaved between score blocks to keep it fed.
  - AV matmul with the ones column producing the softmax denominator as psum
    row 64 (two 1-bank halves for pipelining). Denominator reciprocal ->
    broadcast over 64 partitions via a K=1 ones matmul -> normalize o.
  - proj matmul + (bias + residual) fused in one DVE op -> DMA out.
"""

import numpy as np
import ml_dtypes

import concourse.bass as bass
import concourse.tile as tile
from concourse import bacc, mybir

B, C, HH, WW = 16, 512, 32, 32
N = HH * WW          # 1024 pixels
NH, HD = 8, 64       # heads, head dim
NG, GS = 32, 16      # groups, channels per group
NCORES = 8
BPC = B // NCORES    # batches per core
NT = C // 128        # channel tiles of 128
EPS = 1e-5
SCALE = HD ** -0.5

F32 = mybir.dt.float32
BF16 = mybir.dt.bfloat16


def build_program(qk_bufs=1, out_bufs=2):
    nc = bacc.Bacc(None, target_bir_lowering=False, debug=False)

    x_d = nc.declare_dram_parameter("x", [BPC, 128, NT, N], F32, isOutput=False)
    wqkv_d = nc.declare_dram_parameter("wqkv", [128, NT, 3 * C], BF16, isOutput=False)
    wp_d = nc.declare_dram_parameter("wp", [128, NT, C], BF16, isOutput=False)
    qkvb_d = nc.declare_dram_parameter("qkvb", [128, 2 * NT], F32, isOutput=False)
    vbias_d = nc.declare_dram_parameter("vbias", [128, NH * 65], BF16, isOutput=False)
    pb_d = nc.declare_dram_parameter("pb", [128, NT], F32, isOutput=False)
    nw_d = nc.declare_dram_parameter("nw", [128, NT], F32, isOutput=False)
    nb_d = nc.declare_dram_parameter("nb", [128, NT], F32, isOutput=False)
    gsel_d = nc.declare_dram_parameter("gsel", [128, 128], F32, isOutput=False)
    out_d = nc.declare_dram_parameter("out", [BPC, 128, NT, N], F32, isOutput=True)

    with tile.TileContext(nc) as tc:
        with (
            tc.tile_pool(name="consts", bufs=1) as consts,
            tc.tile_pool(name="xpool", bufs=1) as xpool,
            tc.tile_pool(name="rxpool", bufs=2) as rxpool,
            tc.tile_pool(name="rbpool", bufs=2) as rbpool,
            tc.tile_pool(name="hnpool", bufs=2) as hnpool,
            tc.tile_pool(name="qkpool", bufs=qk_bufs) as qkpool,
            tc.tile_pool(name="vpool", bufs=2) as vpool,
            tc.tile_pool(name="epool", bufs=2) as epool,
            tc.tile_pool(name="opool", bufs=2) as opool,
            tc.tile_pool(name="dpool", bufs=4) as dpool,
            tc.tile_pool(name="outpool", bufs=out_bufs) as outpool,
            tc.tile_pool(name="spool", bufs=2) as spool,
            tc.tile_pool(name="psum", bufs=2, space="PSUM") as psum,
        ):
            # ---- x for batch 0 first: it gates the whole pipeline ----
            x_first = xpool.tile([128, NT, N], F32, name="x_sb")
            for t in range(NT):
                nc.sync.dma_start(out=x_first[:, t, :], in_=x_d[0, :, t, :])

            # ---- constants / weights ----
            wqkv_sb = consts.tile([128, NT, 3 * C], BF16)
            nc.sync.dma_start(out=wqkv_sb, in_=wqkv_d[:])
            wp_sb = consts.tile([128, NT, C], BF16)
            nc.sync.dma_start(out=wp_sb, in_=wp_d[:])
            qkvb_sb = consts.tile([128, 2 * NT], F32)
            nc.sync.dma_start(out=qkvb_sb, in_=qkvb_d[:])
            vbias_sb = consts.tile([128, NH * 65], BF16)
            nc.sync.dma_start(out=vbias_sb, in_=vbias_d[:])
            pb_sb = consts.tile([128, NT], F32)
            nc.sync.dma_start(out=pb_sb, in_=pb_d[:])
            nw_sb = consts.tile([128, NT], F32)
            nc.sync.dma_start(out=nw_sb, in_=nw_d[:])
            nb_sb = consts.tile([128, NT], F32)
            nc.sync.dma_start(out=nb_sb, in_=nb_d[:])
            gsel_sb = consts.tile([128, 128], F32)
            nc.sync.dma_start(out=gsel_sb, in_=gsel_d[:])
            eps_sb = consts.tile([128, 1], F32)
            nc.vector.memset(eps_sb, EPS)
            ones64 = consts.tile([1, 64], BF16)
            nc.vector.memset(ones64, 1.0)

            BNS = nc.vector.BN_STATS_DIM   # 6
            BNA = nc.vector.BN_AGGR_DIM    # 2

            # ---- groupnorm for both batches (all sqrt ACT ops before any exp) ----
            state = {}
            for b in range(BPC):
                if b == 0:
                    x_sb = x_first
                else:
                    x_sb = xpool.tile([128, NT, N], F32, name="x_sb")
                    for t in range(NT):
                        nc.sync.dma_start(out=x_sb[:, t, :], in_=x_d[b, :, t, :])

                stats4 = spool.tile([128, 2 * NT], F32)
                for t in range(NT):
                    bnstat = spool.tile([128, 2, BNS], F32)
                    xv = x_sb[:, t, :].rearrange("p (s n) -> p s n", s=2)
                    for s in range(2):
                        nc.vector.bn_stats(out=bnstat[:, s, :], in_=xv[:, s, :])
                    mv = spool.tile([128, BNA], F32)
                    nc.vector.bn_aggr(out=mv, in_=bnstat)
                    nc.vector.tensor_copy(out=stats4[:, t : t + 1], in_=mv[:, 0:1])
                    nc.vector.scalar_tensor_tensor(
                        out=stats4[:, NT + t : NT + t + 1],
                        in0=mv[:, 0:1],
                        scalar=mv[:, 0:1],
                        in1=mv[:, 1:2],
                        op0=mybir.AluOpType.mult,
                        op1=mybir.AluOpType.add,
                    )

                pst = psum.tile([128, 2 * NT], F32, tag="w", bufs=1)
                nc.tensor.matmul(pst[:], gsel_sb[:], stats4[:], start=True, stop=True)

                mean4 = spool.tile([128, NT], F32)
                nc.vector.tensor_scalar_mul(out=mean4, in0=pst[:, 0:NT], scalar1=1.0 / GS)
                msq4 = spool.tile([128, NT], F32)
                nc.vector.tensor_mul(out=msq4, in0=mean4, in1=mean4)
                var4 = spool.tile([128, NT], F32)
                nc.vector.scalar_tensor_tensor(
                    out=var4,
                    in0=pst[:, NT : 2 * NT],
                    scalar=1.0 / GS,
                    in1=msq4,
                    op0=mybir.AluOpType.mult,
                    op1=mybir.AluOpType.subtract,
                )
                std4 = spool.tile([128, NT], F32)
                nc.scalar.activation(
                    out=std4, in_=var4, func=mybir.ActivationFunctionType.Sqrt,
                    bias=eps_sb[:, 0:1], scale=1.0,
                )
                rstd4 = spool.tile([128, NT], F32)
                nc.vector.reciprocal(out=rstd4, in_=std4)
                a4 = spool.tile([128, NT], F32)
                nc.vector.tensor_mul(out=a4, in0=rstd4, in1=nw_sb)
                mb4 = spool.tile([128, NT], F32)
                nc.vector.tensor_mul(out=mb4, in0=mean4, in1=a4)
                b4 = spool.tile([128, NT], F32)
                nc.vector.tensor_sub(out=b4, in0=nb_sb, in1=mb4)

                hn = hnpool.tile([128, NT, N], BF16)
                for t in range(NT):
                    nc.vector.tensor_scalar(
                        out=hn[:, t, :],
                        in0=x_sb[:, t, :],
                        scalar1=a4[:, t : t + 1],
                        scalar2=b4[:, t : t + 1],
                        op0=mybir.AluOpType.mult,
                        op1=mybir.AluOpType.add,
                    )
                state[b] = {"x": x_sb, "hn": hn}

            # ---- emission helpers (PE queue is in-order: keep ScalarE fed) ----
            def emit_qk_chunk(st, j):
                """q chunk j and k chunk j of qkT (rows j*128 / (NT+j)*128)."""
                hn = st["hn"]
                for rb in (j, NT + j):
                    ps = psum.tile([128, N], F32, tag="w", bufs=1, name="ps_qk")
                    for half in range(2):
                        for kc in range(NT):
                            nc.tensor.matmul(
                                ps[:, half * 512 : (half + 1) * 512],
                                wqkv_sb[:, kc, rb * 128 : (rb + 1) * 128],
                                hn[:, kc, half * 512 : (half + 1) * 512],
                                start=(kc == 0),
                                stop=(kc == NT - 1),
                            )
                    nc.vector.tensor_scalar_add(
                        out=st["qkT"][:, rb, :], in0=ps[:],
                        scalar1=qkvb_sb[:, rb : rb + 1],
                    )

            def emit_proj_rb(st, b, rb, tag="w", bufs=1):
                o_sb = st["o_sb"]
                pp = psum.tile([128, N], F32, tag=tag, bufs=bufs, name="pp")
                for half in range(2):
                    for kc in range(NT):
                        nc.tensor.matmul(
                            pp[:, half * 512 : (half + 1) * 512],
                            wp_sb[:, kc, rb * 128 : (rb + 1) * 128],
                            o_sb[:, kc, half * 512 : (half + 1) * 512],
                            start=(kc == 0),
                            stop=(kc == NT - 1),
                        )
                rx = rxpool.tile([128, N], F32, name="rx")
                nc.sync.dma_start(out=rx, in_=x_d[b, :, rb, :])
                out_sb = outpool.tile([128, N], F32, name="out_sb")
                nc.vector.scalar_tensor_tensor(
                    out=out_sb,
                    in0=pp[:],
                    scalar=pb_sb[:, rb : rb + 1],
                    in1=rx[:],
                    op0=mybir.AluOpType.add,
                    op1=mybir.AluOpType.add,
                )
                nc.sync.dma_start(out=out_d[b, :, rb, :], in_=out_sb[:])

            def emit_scores_unit(st, j, mb, pair):
                """2 concurrent K=64 score matmuls + 2 exps for head pair."""
                pss = [
                    psum.tile([128, N], F32, tag="sc", bufs=2, name="ps_s")
                    for _ in range(2)
                ]
                for half in range(2):
                    for i in range(2):
                        qT_h, kT_h, _ = pair[i]
                        nc.tensor.matmul(
                            pss[i][:, half * 512 : (half + 1) * 512],
                            kT_h[:, mb * 128 : (mb + 1) * 128],
                            qT_h[:, half * 512 : (half + 1) * 512],
                            start=True,
                            stop=True,
                        )
                for i in range(2):
                    nc.scalar.activation(
                        out=pair[i][2][:, mb, :], in_=pss[i][:],
                        func=mybir.ActivationFunctionType.Exp, scale=SCALE,
                    )

            def make_pair(st, j):
                qkT = st["qkT"]
                pair = []
                for i in range(2):
                    h = 2 * j + i
                    poff = (h % 2) * 64
                    qT_h = qkT[poff : poff + 64, h // 2, :]
                    kT_h = qkT[poff : poff + 64, NT + h // 2, :]
                    expT = epool.tile([128, 8, N], BF16, name="expT", tag=f"expT{i}")
                    pair.append((qT_h, kT_h, expT))
                return pair

            def av_units(st, j, pair):
                """AV + normalize for pair j as a list of small PE/DVE units."""
                v_pad = st["v_pad"]
                o_sb = st["o_sb"]
                units = []
                for i in range(2):
                    h = 2 * j + i
                    poff = (h % 2) * 64
                    expT = pair[i][2]
                    box = {}

                    def chain(h=h, expT=expT, half=0, box=box):
                        po = psum.tile([65, 512], F32, tag="o", bufs=2, name="po")
                        for mb in range(8):
                            nc.tensor.matmul(
                                po[:],
                                v_pad[:, mb, h * 65 : (h + 1) * 65],
                                expT[:, mb, half * 512 : (half + 1) * 512],
                                start=(mb == 0),
                                stop=(mb == 7),
                            )
                        denb = dpool.tile([1, 512], BF16, name="denb")
                        with nc.allow_low_precision(reason="softmax denom bf16"):
                            nc.vector.tensor_copy(out=denb, in_=po[64:65, :])
                        box[("po", half)] = po
                        box[("denb", half)] = denb

                    def finish(h=h, poff=poff, box=box):
                        pbc = psum.tile([64, N], F32, tag="w", bufs=1, name="pbc")
                        for half in range(2):
                            nc.tensor.matmul(
                                pbc[:, half * 512 : (half + 1) * 512],
                                ones64[:],
                                box[("denb", half)][:],
                                start=True,
                                stop=True,
                            )
                        recip_bc = rbpool.tile([64, N], F32, name="recip_bc")
                        nc.vector.reciprocal_approx_fast(out=recip_bc, in_=pbc[:])
                        for half in range(2):
                            nc.vector.tensor_mul(
                                out=o_sb[
                                    poff : poff + 64, h // 2,
                                    half * 512 : (half + 1) * 512,
                                ],
                                in0=box[("po", half)][0:64, :],
                                in1=recip_bc[:, half * 512 : (half + 1) * 512],
                            )

                    units.append(lambda c=chain: c(half=0))
                    units.append(lambda c=chain: c(half=1))
                    units.append(finish)
                return units

            def qk_units(st, j, tag="w", bufs=1):
                us = []
                for rb in (j, NT + j):
                    def u(st=st, rb=rb):
                        hn = st["hn"]
                        ps = psum.tile([128, N], F32, tag=tag, bufs=bufs, name="ps_qk")
                        for half in range(2):
                            for kc in range(NT):
                                nc.tensor.matmul(
                                    ps[:, half * 512 : (half + 1) * 512],
                                    wqkv_sb[:, kc, rb * 128 : (rb + 1) * 128],
                                    hn[:, kc, half * 512 : (half + 1) * 512],
                                    start=(kc == 0),
                                    stop=(kc == NT - 1),
                                )
                        nc.vector.tensor_scalar_add(
                            out=st["qkT"][:, rb, :], in0=ps[:],
                            scalar1=qkvb_sb[:, rb : rb + 1],
                        )
                    us.append(u)
                return us

            def v_unit(st, mbp):
                def u(st=st, mbp=mbp):
                    hn = st["hn"]
                    v_pad = st["v_pad"]
                    psv = psum.tile([128, N], F32, tag="w", bufs=1, name="psv")
                    for half in range(2):
                        mb = 2 * mbp + half
                        for kc in range(NT):
                            nc.tensor.matmul(
                                psv[:, half * 512 : (half + 1) * 512],
                                hn[:, kc, mb * 128 : (mb + 1) * 128],
                                wqkv_sb[:, kc, 2 * C : 3 * C],
                                start=(kc == 0),
                                stop=(kc == NT - 1),
                            )
                        nc.vector.tensor_tensor(
                            out=v_pad[:, mb, :].rearrange("p (h c) -> p h c", c=65)[
                                :, :, 0:64
                            ],
                            in0=psv[:, half * 512 : (half + 1) * 512].rearrange(
                                "p (h c) -> p h c", c=64
                            ),
                            in1=vbias_sb.rearrange("p (h c) -> p h c", c=65)[
                                :, :, 0:64
                            ],
                            op=mybir.AluOpType.add,
                        )
                return u

            def proj_unit(st, b, rb):
                def u():
                    emit_proj_rb(st, b, rb)
                return u

            def setup_batch(b):
                st = state[b]
                st["qkT"] = qkpool.tile([128, 2 * NT, N], BF16, name="qkT")
                st["v_pad"] = vpool.tile([128, 8, NH * 65], BF16, name="v_pad")
                ones_view = st["v_pad"].rearrange("p m (h c) -> p m h c", c=65)[
                    :, :, :, 64:65
                ]
                nc.vector.memset(ones_view, 1.0)
                st["o_sb"] = opool.tile([128, NT, N], BF16, name="o_sb")

            setup_batch(0)
            for u in qk_units(state[0], 0, tag="sc", bufs=2):
                u()

            def fillers(b, j):
                s0, s1 = state[0], state.get(1)
                table = {
                    (0, 0): [v_unit(s0, 0), v_unit(s0, 1), v_unit(s0, 2),
                             v_unit(s0, 3)] + qk_units(s0, 1),
                    (0, 1): qk_units(s0, 2),
                    (0, 2): qk_units(s0, 3),
                    (0, 3): qk_units(s1, 0) + [v_unit(s1, 0), v_unit(s1, 1)],
                    (1, 0): qk_units(s1, 1) + [v_unit(s1, 2), v_unit(s1, 3)],
                    (1, 1): qk_units(s1, 2) + [proj_unit(s0, 0, 0),
                                               proj_unit(s0, 0, 1)],
                    (1, 2): qk_units(s1, 3) + [proj_unit(s0, 0, 2),
                                               proj_unit(s0, 0, 3)],
                    (1, 3): [],
                }
                return table[(b, j)]

            pending = None
            for b in range(BPC):
                st = state[b]
                for j in range(4):
                    if (b, j) == (0, 3):
                        setup_batch(1)
                    pair = make_pair(st, j)
                    av = av_units(*pending) if pending is not None else []
                    fil = fillers(b, j)
                    units = []
                    while av or fil:
                        if av:
                            units.append(av.pop(0))
                        if fil:
                            units.append(fil.pop(0))
                    pending = (st, j, pair)
                    k = 0
                    for mb in range(8):
                        emit_scores_unit(st, j, mb, pair)
                        target = (mb + 1) * len(units) // 8
                        while k < target:
                            units[k]()
                            k += 1

            for u in av_units(*pending):
                u()
            for rb in range(NT):
                emit_proj_rb(state[BPC - 1], BPC - 1, rb, tag="sc", bufs=2)

    nc.finalize()
    return nc


_PROGRAM = None


def _get_program():
    global _PROGRAM
    if _PROGRAM is None:
        _PROGRAM = build_program()
    return _PROGRAM


def _prep_inputs(x, norm_w, norm_b, qkv_w, qkv_b, proj_w, proj_b):
    x = np.asarray(x, np.float32)
    xs = np.ascontiguousarray(
        x.reshape(B, NT, 128, N).transpose(0, 2, 1, 3)
    )  # (B, 128, NT, N)

    wqkvT = np.asarray(qkv_w, np.float32).T  # (C, 3C)
    wqkv = np.ascontiguousarray(
        wqkvT.reshape(NT, 128, 3 * C).transpose(1, 0, 2)
    ).astype(ml_dtypes.bfloat16)
    wpT = np.asarray(proj_w, np.float32).T
    wp = np.ascontiguousarray(wpT.reshape(NT, 128, C).transpose(1, 0, 2)).astype(
        ml_dtypes.bfloat16
    )

    qkv_b = np.asarray(qkv_b, np.float32)
    qkvb8 = np.ascontiguousarray(qkv_b[: 2 * C].reshape(2 * NT, 128).T)  # (128, 8)
    vb = np.zeros((NH, 65), np.float32)
    vb[:, :64] = qkv_b[2 * C :].reshape(NH, 64)
    vbias = np.ascontiguousarray(
        np.broadcast_to(vb.reshape(1, NH * 65), (128, NH * 65))
    )
    pb4 = np.ascontiguousarray(np.asarray(proj_b, np.float32).reshape(NT, 128).T)
    nw4 = np.ascontiguousarray(np.asarray(norm_w, np.float32).reshape(NT, 128).T)
    nb4 = np.ascontiguousarray(np.asarray(norm_b, np.float32).reshape(NT, 128).T)

    idx = np.arange(128)
    gsel = (idx[:, None] // GS == idx[None, :] // GS).astype(np.float32)

    shared = {
        "wqkv": wqkv, "wp": wp, "qkvb": qkvb8, "vbias": vbias, "pb": pb4,
        "nw": nw4, "nb": nb4, "gsel": gsel,
    }
    in_maps = [
        {"x": np.ascontiguousarray(xs[c * BPC : (c + 1) * BPC]), **shared}
        for c in range(NCORES)
    ]
    return in_maps


def _assemble(results):
    outs = np.concatenate(
        [results[c]["out"] for c in range(NCORES)], axis=0
    )  # (B, 128, NT, N)
    return np.ascontiguousarray(
        outs.transpose(0, 2, 1, 3).reshape(B, C, HH, WW)
    ).astype(np.float32)


def kernel(x, norm_w, norm_b, qkv_w, qkv_b, proj_w, proj_b, _trace=False):
    from concourse.bass_utils import run_bass_kernel_spmd

    nc = _get_program()
    in_maps = _prep_inputs(x, norm_w, norm_b, qkv_w, qkv_b, proj_w, proj_b)
    res = run_bass_kernel_spmd(nc, in_maps, list(range(NCORES)), trace=_trace)
    out = _assemble(res.results)
    if _trace:
        return out, res
    return out


# revision 33
# speedup vs baseline: 1.3967x; 1.0133x over previous
"""AttentionBlock (GroupNorm + 8-head self-attention + proj + residual) on 8 trn2 cores.

Sharding: data-parallel over batch (16 batches -> 2 per core), no collectives.

Per-core device program (per batch):
  - GroupNorm(32, 512): bn_stats per 128-channel tile -> per-channel [mean, E[x^2]]
    -> cross-partition group reduce via a (128,128) group-indicator fp32 matmul
    -> per-channel scale/bias -> hn (bf16).
  - QKV 1x1 conv as matmuls (bf16): q,k produced in (channel, pixel) layout;
    v produced in (pixel, channel) layout, padded with a ones column per head.
  - Attention head-pair j=(2j, 2j+1): the two heads sit at partition offsets
    0/64 of the same qkT chunk, so their K=64 score matmuls occupy disjoint
    PE row-groups (tile_position auto-derived from base partition) and run
    concurrently. exp on ScalarE straight from PSUM (scores bounded ~6.5 ->
    no max subtraction); ScalarE is the kernel bottleneck, so qkv/v/proj
    matmul work is interleaved between score blocks to keep it fed.
  - AV matmul with the ones column producing the softmax denominator as psum
    row 64 (two 1-bank halves for pipelining). Denominator reciprocal ->
    broadcast over 64 partitions via a K=1 ones matmul -> normalize o.
  - proj matmul + (bias + residual) fused in one DVE op -> DMA out.
"""

import numpy as np
import ml_dtypes

import concourse.bass as bass
import concourse.tile as tile
from concourse import bacc, mybir

B, C, HH, WW = 16, 512, 32, 32
N = HH * WW          # 1024 pixels
NH, HD = 8, 64       # heads, head dim
NG, GS = 32, 16      # groups, channels per group
NCORES = 8
BPC = B // NCORES    # batches per core
NT = C // 128        # channel tiles of 128
EPS = 1e-5
SCALE = HD ** -0.5

F32 = mybir.dt.float32
BF16 = mybir.dt.bfloat16


def build_program(qk_bufs=1, out_bufs=2):
    nc = bacc.Bacc(None, target_bir_lowering=False, debug=False)

    x_d = nc.declare_dram_parameter("x", [BPC, 128, NT, N], F32, isOutput=False)
    wqkv_d = nc.declare_dram_parameter("wqkv", [128, NT, 3 * C], BF16, isOutput=False)
    wp_d = nc.declare_dram_parameter("wp", [128, NT, C], BF16, isOutput=False)
    qkvb_d = nc.declare_dram_parameter("qkvb", [128, 2 * NT], F32, isOutput=False)
    vbias_d = nc.declare_dram_parameter("vbias", [128, NH * 65], BF16, isOutput=False)
    pb_d = nc.declare_dram_parameter("pb", [128, NT], F32, isOutput=False)
    nw_d = nc.declare_dram_parameter("nw", [128, NT], F32, isOutput=False)
    nb_d = nc.declare_dram_parameter("nb", [128, NT], F32, isOutput=False)
    gsel_d = nc.declare_dram_parameter("gsel", [128, 128], F32, isOutput=False)
    out_d = nc.declare_dram_parameter("out", [BPC, 128, NT, N], F32, isOutput=True)

    with tile.TileContext(nc) as tc:
        with (
            tc.tile_pool(name="consts", bufs=1) as consts,
            tc.tile_pool(name="xpool", bufs=1) as xpool,
            tc.tile_pool(name="rxpool", bufs=2) as rxpool,
            tc.tile_pool(name="rbpool", bufs=2) as rbpool,
            tc.tile_pool(name="hnpool", bufs=2) as hnpool,
            tc.tile_pool(name="qkpool", bufs=qk_bufs) as qkpool,
            tc.tile_pool(name="vpool", bufs=2) as vpool,
            tc.tile_pool(name="epool", bufs=2) as epool,
            tc.tile_pool(name="opool", bufs=2) as opool,
            tc.tile_pool(name="dpool", bufs=4) as dpool,
            tc.tile_pool(name="outpool", bufs=out_bufs) as outpool,
            tc.tile_pool(name="spool", bufs=2) as spool,
            tc.tile_pool(name="psum", bufs=2, space="PSUM") as psum,
        ):
            # ---- x for batch 0 first: it gates the whole pipeline ----
            x_first = xpool.tile([128, NT, N], F32, name="x_sb")
            for t in range(NT):
                nc.sync.dma_start(out=x_first[:, t, :], in_=x_d[0, :, t, :])

            # ---- constants / weights ----
            wqkv_sb = consts.tile([128, NT, 3 * C], BF16)
            nc.sync.dma_start(out=wqkv_sb, in_=wqkv_d[:])
            wp_sb = consts.tile([128, NT, C], BF16)
            nc.sync.dma_start(out=wp_sb, in_=wp_d[:])
            qkvb_sb = consts.tile([128, 2 * NT], F32)
            nc.sync.dma_start(out=qkvb_sb, in_=qkvb_d[:])
            vbias_sb = consts.tile([128, NH * 65], BF16)
            nc.sync.dma_start(out=vbias_sb, in_=vbias_d[:])
            pb_sb = consts.tile([128, NT], F32)
            nc.sync.dma_start(out=pb_sb, in_=pb_d[:])
            nw_sb = consts.tile([128, NT], F32)
            nc.sync.dma_start(out=nw_sb, in_=nw_d[:])
            nb_sb = consts.tile([128, NT], F32)
            nc.sync.dma_start(out=nb_sb, in_=nb_d[:])
            gsel_sb = consts.tile([128, 128], F32)
            nc.sync.dma_start(out=gsel_sb, in_=gsel_d[:])
            eps_sb = consts.tile([128, 1], F32)
            nc.vector.memset(eps_sb, EPS)
            ones64 = consts.tile([1, 64], BF16)
            nc.vector.memset(ones64, 1.0)
            warm = consts.tile([1, 1], F32)
            nc.scalar.activation(
                out=warm, in_=eps_sb[0:1, 0:1],
                func=mybir.ActivationFunctionType.Exp, scale=1.0,
            )

            BNS = nc.vector.BN_STATS_DIM   # 6
            BNA = nc.vector.BN_AGGR_DIM    # 2

            # ---- groupnorm for both batches (all sqrt ACT ops before any exp) ----
            state = {}
            for b in range(BPC):
                if b == 0:
                    x_sb = x_first
                else:
                    x_sb = xpool.tile([128, NT, N], F32, name="x_sb")
                    for t in range(NT):
                        nc.sync.dma_start(out=x_sb[:, t, :], in_=x_d[b, :, t, :])

                stats4 = spool.tile([128, 2 * NT], F32)
                for t in range(NT):
                    bnstat = spool.tile([128, 2, BNS], F32)
                    xv = x_sb[:, t, :].rearrange("p (s n) -> p s n", s=2)
                    for s in range(2):
                        nc.vector.bn_stats(out=bnstat[:, s, :], in_=xv[:, s, :])
                    mv = spool.tile([128, BNA], F32)
                    nc.vector.bn_aggr(out=mv, in_=bnstat)
                    nc.vector.tensor_copy(out=stats4[:, t : t + 1], in_=mv[:, 0:1])
                    nc.vector.scalar_tensor_tensor(
                        out=stats4[:, NT + t : NT + t + 1],
                        in0=mv[:, 0:1],
                        scalar=mv[:, 0:1],
                        in1=mv[:, 1:2],
                        op0=mybir.AluOpType.mult,
                        op1=mybir.AluOpType.add,
                    )

                pst = psum.tile([128, 2 * NT], F32, tag="w", bufs=1)
                nc.tensor.matmul(pst[:], gsel_sb[:], stats4[:], start=True, stop=True)

                mean4 = spool.tile([128, NT], F32)
                nc.vector.tensor_scalar_mul(out=mean4, in0=pst[:, 0:NT], scalar1=1.0 / GS)
                msq4 = spool.tile([128, NT], F32)
                nc.vector.tensor_mul(out=msq4, in0=mean4, in1=mean4)
                var4 = spool.tile([128, NT], F32)
                nc.vector.scalar_tensor_tensor(
                    out=var4,
                    in0=pst[:, NT : 2 * NT],
                    scalar=1.0 / GS,
                    in1=msq4,
                    op0=mybir.AluOpType.mult,
                    op1=mybir.AluOpType.subtract,
                )
                # rstd = 1/sqrt(var + eps), Newton on DVE (keeps ScalarE
                # exp-only so its activation table never swaps)
                ve = spool.tile([128, NT], F32)
                nc.vector.tensor_scalar_add(out=ve, in0=var4, scalar1=EPS)
                vi = ve.bitcast(mybir.dt.int32)
                sh = spool.tile([128, NT], mybir.dt.int32)
                nc.vector.tensor_scalar(
                    out=sh, in0=vi, scalar1=1, scalar2=-1,
                    op0=mybir.AluOpType.arith_shift_right,
                    op1=mybir.AluOpType.bitwise_xor,
                )
                y0i = spool.tile([128, NT], mybir.dt.int32)
                nc.vector.tensor_scalar_add(out=y0i, in0=sh, scalar1=0x5F3759E0)
                rstd4 = y0i.bitcast(F32)
                for _ in range(2):
                    yy = spool.tile([128, NT], F32)
                    nc.vector.tensor_mul(out=yy, in0=rstd4, in1=rstd4)
                    vyy = spool.tile([128, NT], F32)
                    nc.vector.tensor_mul(out=vyy, in0=ve, in1=yy)
                    w = spool.tile([128, NT], F32)
                    nc.vector.tensor_scalar(
                        out=w, in0=vyy, scalar1=-0.5, scalar2=1.5,
                        op0=mybir.AluOpType.mult, op1=mybir.AluOpType.add,
                    )
                    rs2 = spool.tile([128, NT], F32)
                    nc.vector.tensor_mul(out=rs2, in0=rstd4, in1=w)
                    rstd4 = rs2
                a4 = spool.tile([128, NT], F32)
                nc.vector.tensor_mul(out=a4, in0=rstd4, in1=nw_sb)
                mb4 = spool.tile([128, NT], F32)
                nc.vector.tensor_mul(out=mb4, in0=mean4, in1=a4)
                b4 = spool.tile([128, NT], F32)
                nc.vector.tensor_sub(out=b4, in0=nb_sb, in1=mb4)

                hn = hnpool.tile([128, NT, N], BF16)
                for t in range(NT):
                    nc.vector.tensor_scalar(
                        out=hn[:, t, :],
                        in0=x_sb[:, t, :],
                        scalar1=a4[:, t : t + 1],
                        scalar2=b4[:, t : t + 1],
                        op0=mybir.AluOpType.mult,
                        op1=mybir.AluOpType.add,
                    )
                state[b] = {"x": x_sb, "hn": hn}

            # ---- emission helpers (PE queue is in-order: keep ScalarE fed) ----
            def emit_qk_chunk(st, j):
                """q chunk j and k chunk j of qkT (rows j*128 / (NT+j)*128)."""
                hn = st["hn"]
                for rb in (j, NT + j):
                    ps = psum.tile([128, N], F32, tag="w", bufs=1, name="ps_qk")
                    for half in range(2):
                        for kc in range(NT):
                            nc.tensor.matmul(
                                ps[:, half * 512 : (half + 1) * 512],
                                wqkv_sb[:, kc, rb * 128 : (rb + 1) * 128],
                                hn[:, kc, half * 512 : (half + 1) * 512],
                                start=(kc == 0),
                                stop=(kc == NT - 1),
                            )
                    nc.vector.tensor_scalar_add(
                        out=st["qkT"][:, rb, :], in0=ps[:],
                        scalar1=qkvb_sb[:, rb : rb + 1],
                    )

            def emit_proj_rb(st, b, rb, tag="w", bufs=1):
                o_sb = st["o_sb"]
                pp = psum.tile([128, N], F32, tag=tag, bufs=bufs, name="pp")
                for half in range(2):
                    for kc in range(NT):
                        nc.tensor.matmul(
                            pp[:, half * 512 : (half + 1) * 512],
                            wp_sb[:, kc, rb * 128 : (rb + 1) * 128],
                            o_sb[:, kc, half * 512 : (half + 1) * 512],
                            start=(kc == 0),
                            stop=(kc == NT - 1),
                        )
                rx = rxpool.tile([128, N], F32, name="rx")
                nc.sync.dma_start(out=rx, in_=x_d[b, :, rb, :])
                out_sb = outpool.tile([128, N], F32, name="out_sb")
                nc.vector.scalar_tensor_tensor(
                    out=out_sb,
                    in0=pp[:],
                    scalar=pb_sb[:, rb : rb + 1],
                    in1=rx[:],
                    op0=mybir.AluOpType.add,
                    op1=mybir.AluOpType.add,
                )
                nc.sync.dma_start(out=out_d[b, :, rb, :], in_=out_sb[:])

            def emit_scores_unit(st, j, mb, pair):
                """2 concurrent K=64 score matmuls + 2 exps for head pair."""
                pss = [
                    psum.tile([128, N], F32, tag="sc", bufs=2, name="ps_s")
                    for _ in range(2)
                ]
                for half in range(2):
                    for i in range(2):
                        qT_h, kT_h, _ = pair[i]
                        nc.tensor.matmul(
                            pss[i][:, half * 512 : (half + 1) * 512],
                            kT_h[:, mb * 128 : (mb + 1) * 128],
                            qT_h[:, half * 512 : (half + 1) * 512],
                            start=True,
                            stop=True,
                            tile_position=(i * 64, 0),
                        )
                for i in range(2):
                    nc.scalar.activation(
                        out=pair[i][2][:, mb, :], in_=pss[i][:],
                        func=mybir.ActivationFunctionType.Exp, scale=SCALE,
                    )

            def make_pair(st, j):
                qkT = st["qkT"]
                pair = []
                for i in range(2):
                    h = 2 * j + i
                    poff = (h % 2) * 64
                    qT_h = qkT[poff : poff + 64, h // 2, :]
                    kT_h = qkT[poff : poff + 64, NT + h // 2, :]
                    expT = epool.tile([128, 8, N], BF16, name="expT", tag=f"expT{i}")
                    pair.append((qT_h, kT_h, expT))
                return pair

            def av_units(st, j, pair, po_tags=("o", "o")):
                """AV + normalize for pair j as a list of small PE/DVE units."""
                v_pad = st["v_pad"]
                o_sb = st["o_sb"]
                units = []
                for i in range(2):
                    h = 2 * j + i
                    poff = (h % 2) * 64
                    expT = pair[i][2]
                    box = {}
                    po_tag = po_tags[i]

                    def chain(h=h, expT=expT, half=0, box=box, po_tag=po_tag):
                        po = psum.tile([65, 512], F32, tag=po_tag, bufs=2, name="po")
                        for mb in range(8):
                            nc.tensor.matmul(
                                po[:],
                                v_pad[:, mb, h * 65 : (h + 1) * 65],
                                expT[:, mb, half * 512 : (half + 1) * 512],
                                start=(mb == 0),
                                stop=(mb == 7),
                            )
                        denb = dpool.tile([1, 512], BF16, name="denb")
                        with nc.allow_low_precision(reason="softmax denom bf16"):
                            nc.vector.tensor_copy(out=denb, in_=po[64:65, :])
                        box[("po", half)] = po
                        box[("denb", half)] = denb

                    def finish(h=h, poff=poff, box=box):
                        pbc = psum.tile([64, N], F32, tag="w", bufs=1, name="pbc")
                        for half in range(2):
                            nc.tensor.matmul(
                                pbc[:, half * 512 : (half + 1) * 512],
                                ones64[:],
                                box[("denb", half)][:],
                                start=True,
                                stop=True,
                            )
                        recip_bc = rbpool.tile([64, N], F32, name="recip_bc")
                        nc.vector.reciprocal_approx_fast(out=recip_bc, in_=pbc[:])
                        for half in range(2):
                            nc.vector.tensor_mul(
                                out=o_sb[
                                    poff : poff + 64, h // 2,
                                    half * 512 : (half + 1) * 512,
                                ],
                                in0=box[("po", half)][0:64, :],
                                in1=recip_bc[:, half * 512 : (half + 1) * 512],
                            )

                    units.append(lambda c=chain: c(half=0))
                    units.append(lambda c=chain: c(half=1))
                    units.append(finish)
                return units

            def qk_units(st, j, tag="w", bufs=1):
                us = []
                for rb in (j, NT + j):
                    def u(st=st, rb=rb):
                        hn = st["hn"]
                        ps = psum.tile([128, N], F32, tag=tag, bufs=bufs, name="ps_qk")
                        for half in range(2):
                            for kc in range(NT):
                                nc.tensor.matmul(
                                    ps[:, half * 512 : (half + 1) * 512],
                                    wqkv_sb[:, kc, rb * 128 : (rb + 1) * 128],
                                    hn[:, kc, half * 512 : (half + 1) * 512],
                                    start=(kc == 0),
                                    stop=(kc == NT - 1),
                                )
                        nc.vector.tensor_scalar_add(
                            out=st["qkT"][:, rb, :], in0=ps[:],
                            scalar1=qkvb_sb[:, rb : rb + 1],
                        )
                    us.append(u)
                return us

            def v_unit(st, mbp):
                def u(st=st, mbp=mbp):
                    hn = st["hn"]
                    v_pad = st["v_pad"]
                    psv = psum.tile([128, N], F32, tag="w", bufs=1, name="psv")
                    for half in range(2):
                        mb = 2 * mbp + half
                        for kc in range(NT):
                            nc.tensor.matmul(
                                psv[:, half * 512 : (half + 1) * 512],
                                hn[:, kc, mb * 128 : (mb + 1) * 128],
                                wqkv_sb[:, kc, 2 * C : 3 * C],
                                start=(kc == 0),
                                stop=(kc == NT - 1),
                            )
                        nc.vector.tensor_tensor(
                            out=v_pad[:, mb, :].rearrange("p (h c) -> p h c", c=65)[
                                :, :, 0:64
                            ],
                            in0=psv[:, half * 512 : (half + 1) * 512].rearrange(
                                "p (h c) -> p h c", c=64
                            ),
                            in1=vbias_sb.rearrange("p (h c) -> p h c", c=65)[
                                :, :, 0:64
                            ],
                            op=mybir.AluOpType.add,
                        )
                return u

            def proj_unit(st, b, rb):
                def u():
                    emit_proj_rb(st, b, rb)
                return u

            def setup_batch(b):
                st = state[b]
                st["qkT"] = qkpool.tile([128, 2 * NT, N], BF16, name="qkT")
                st["v_pad"] = vpool.tile([128, 8, NH * 65], BF16, name="v_pad")
                ones_view = st["v_pad"].rearrange("p m (h c) -> p m h c", c=65)[
                    :, :, :, 64:65
                ]
                nc.vector.memset(ones_view, 1.0)
                st["o_sb"] = opool.tile([128, NT, N], BF16, name="o_sb")

            setup_batch(0)
            for u in qk_units(state[0], 0, tag="sc", bufs=2):
                u()

            def fillers(b, j):
                s0, s1 = state[0], state.get(1)
                table = {
                    (0, 0): [v_unit(s0, 0), v_unit(s0, 1), v_unit(s0, 2),
                             v_unit(s0, 3)] + qk_units(s0, 1),
                    (0, 1): qk_units(s0, 2),
                    (0, 2): qk_units(s0, 3),
                    (0, 3): qk_units(s1, 0) + [v_unit(s1, 0), v_unit(s1, 1)],
                    (1, 0): qk_units(s1, 1) + [v_unit(s1, 2), v_unit(s1, 3)],
                    (1, 1): qk_units(s1, 2) + [proj_unit(s0, 0, 0),
                                               proj_unit(s0, 0, 1)],
                    (1, 2): qk_units(s1, 3) + [proj_unit(s0, 0, 2),
                                               proj_unit(s0, 0, 3)],
                    (1, 3): [],
                }
                return table[(b, j)]

            pending = None
            for b in range(BPC):
                st = state[b]
                for j in range(4):
                    if (b, j) == (0, 3):
                        setup_batch(1)
                    pair = make_pair(st, j)
                    av = av_units(*pending) if pending is not None else []
                    fil = fillers(b, j)
                    units = []
                    while av or fil:
                        if av:
                            units.append(av.pop(0))
                        if fil:
                            units.append(fil.pop(0))
                    pending = (st, j, pair)
                    k = 0
                    for mb in range(8):
                        emit_scores_unit(st, j, mb, pair)
                        target = (mb + 1) * len(units) // 8
                        while k < target:
                            units[k]()
                            k += 1

            for u in av_units(*pending, po_tags=("o", "sc")):
                u()
            for rb in range(NT):
                emit_proj_rb(state[BPC - 1], BPC - 1, rb, tag="sc", bufs=2)

    nc.finalize()
    return nc


_PROGRAM = None


def _get_program():
    global _PROGRAM
    if _PROGRAM is None:
        _PROGRAM = build_program()
    return _PROGRAM


def _prep_inputs(x, norm_w, norm_b, qkv_w, qkv_b, proj_w, proj_b):
    x = np.asarray(x, np.float32)
    xs = np.ascontiguousarray(
        x.reshape(B, NT, 128, N).transpose(0, 2, 1, 3)
    )  # (B, 128, NT, N)

    wqkvT = np.asarray(qkv_w, np.float32).T  # (C, 3C)
    wqkv = np.ascontiguousarray(
        wqkvT.reshape(NT, 128, 3 * C).transpose(1, 0, 2)
    ).astype(ml_dtypes.bfloat16)
    wpT = np.asarray(proj_w, np.float32).T
    wp = np.ascontiguousarray(wpT.reshape(NT, 128, C).transpose(1, 0, 2)).astype(
        ml_dtypes.bfloat16
    )

    qkv_b = np.asarray(qkv_b, np.float32)
    qkvb8 = np.ascontiguousarray(qkv_b[: 2 * C].reshape(2 * NT, 128).T)  # (128, 8)
    vb = np.zeros((NH, 65), np.float32)
    vb[:, :64] = qkv_b[2 * C :].reshape(NH, 64)
    vbias = np.ascontiguousarray(
        np.broadcast_to(vb.reshape(1, NH * 65), (128, NH * 65))
    )
    pb4 = np.ascontiguousarray(np.asarray(proj_b, np.float32).reshape(NT, 128).T)
    nw4 = np.ascontiguousarray(np.asarray(norm_w, np.float32).reshape(NT, 128).T)
    nb4 = np.ascontiguousarray(np.asarray(norm_b, np.float32).reshape(NT, 128).T)

    idx = np.arange(128)
    gsel = (idx[:, None] // GS == idx[None, :] // GS).astype(np.float32)

    shared = {
        "wqkv": wqkv, "wp": wp, "qkvb": qkvb8, "vbias": vbias, "pb": pb4,
        "nw": nw4, "nb": nb4, "gsel": gsel,
    }
    in_maps = [
        {"x": np.ascontiguousarray(xs[c * BPC : (c + 1) * BPC]), **shared}
        for c in range(NCORES)
    ]
    return in_maps


def _assemble(results):
    outs = np.concatenate(
        [results[c]["out"] for c in range(NCORES)], axis=0
    )  # (B, 128, NT, N)
    return np.ascontiguousarray(
        outs.transpose(0, 2, 1, 3).reshape(B, C, HH, WW)
    ).astype(np.float32)


def kernel(x, norm_w, norm_b, qkv_w, qkv_b, proj_w, proj_b, _trace=False):
    from concourse.bass_utils import run_bass_kernel_spmd

    nc = _get_program()
    in_maps = _prep_inputs(x, norm_w, norm_b, qkv_w, qkv_b, proj_w, proj_b)
    res = run_bass_kernel_spmd(nc, in_maps, list(range(NCORES)), trace=_trace)
    out = _assemble(res.results)
    if _trace:
        return out, res
    return out


# revision 34
# speedup vs baseline: 1.4134x; 1.0119x over previous
"""AttentionBlock (GroupNorm + 8-head self-attention + proj + residual) on 8 trn2 cores.

Sharding: data-parallel over batch (16 batches -> 2 per core), no collectives.

Per-core device program (per batch):
  - GroupNorm(32, 512): bn_stats per 128-channel tile -> per-channel [mean, E[x^2]]
    -> cross-partition group reduce via a (128,128) group-indicator fp32 matmul
    -> per-channel scale/bias -> hn (bf16).
  - QKV 1x1 conv as matmuls (bf16): q,k produced in (channel, pixel) layout;
    v produced in (pixel, channel) layout, padded with a ones column per head.
  - Attention head-pair j=(2j, 2j+1): the two heads sit at partition offsets
    0/64 of the same qkT chunk, so their K=64 score matmuls occupy disjoint
    PE row-groups (tile_position auto-derived from base partition) and run
    concurrently. exp on ScalarE straight from PSUM (scores bounded ~6.5 ->
    no max subtraction); ScalarE is the kernel bottleneck, so qkv/v/proj
    matmul work is interleaved between score blocks to keep it fed.
  - AV matmul with the ones column producing the softmax denominator as psum
    row 64 (two 1-bank halves for pipelining). Denominator reciprocal ->
    broadcast over 64 partitions via a K=1 ones matmul -> normalize o.
  - proj matmul + (bias + residual) fused in one DVE op -> DMA out.
"""

import numpy as np
import ml_dtypes

import concourse.bass as bass
import concourse.tile as tile
from concourse import bacc, mybir

B, C, HH, WW = 16, 512, 32, 32
N = HH * WW          # 1024 pixels
NH, HD = 8, 64       # heads, head dim
NG, GS = 32, 16      # groups, channels per group
NCORES = 8
BPC = B // NCORES    # batches per core
NT = C // 128        # channel tiles of 128
EPS = 1e-5
SCALE = HD ** -0.5

F32 = mybir.dt.float32
BF16 = mybir.dt.bfloat16


def build_program(qk_bufs=1, out_bufs=2):
    nc = bacc.Bacc(None, target_bir_lowering=False, debug=False)

    x_d = nc.declare_dram_parameter("x", [BPC, 128, NT, N], F32, isOutput=False)
    wqkv_d = nc.declare_dram_parameter("wqkv", [128, NT, 3 * C], BF16, isOutput=False)
    wp_d = nc.declare_dram_parameter("wp", [128, NT, C], BF16, isOutput=False)
    qkvb_d = nc.declare_dram_parameter("qkvb", [128, 2 * NT], F32, isOutput=False)
    vbias_d = nc.declare_dram_parameter("vbias", [128, NH * 65], BF16, isOutput=False)
    pb_d = nc.declare_dram_parameter("pb", [128, NT], F32, isOutput=False)
    nw_d = nc.declare_dram_parameter("nw", [128, NT], F32, isOutput=False)
    nb_d = nc.declare_dram_parameter("nb", [128, NT], F32, isOutput=False)
    gsel_d = nc.declare_dram_parameter("gsel", [128, 128], F32, isOutput=False)
    out_d = nc.declare_dram_parameter("out", [BPC, 128, NT, N], F32, isOutput=True)

    with tile.TileContext(nc) as tc:
        with (
            tc.tile_pool(name="consts", bufs=1) as consts,
            tc.tile_pool(name="xpool", bufs=1) as xpool,
            tc.tile_pool(name="rxpool", bufs=2) as rxpool,
            tc.tile_pool(name="rbpool", bufs=2) as rbpool,
            tc.tile_pool(name="hnpool", bufs=2) as hnpool,
            tc.tile_pool(name="qkpool", bufs=qk_bufs) as qkpool,
            tc.tile_pool(name="vpool", bufs=2) as vpool,
            tc.tile_pool(name="epool", bufs=2) as epool,
            tc.tile_pool(name="opool", bufs=2) as opool,
            tc.tile_pool(name="dpool", bufs=4) as dpool,
            tc.tile_pool(name="outpool", bufs=out_bufs) as outpool,
            tc.tile_pool(name="spool", bufs=2) as spool,
            tc.tile_pool(name="psum", bufs=2, space="PSUM") as psum,
        ):
            # ---- x for batch 0 first: it gates the whole pipeline ----
            x_first = xpool.tile([128, NT, N], F32, name="x_sb")
            for t in range(NT):
                nc.sync.dma_start(out=x_first[:, t, :], in_=x_d[0, :, t, :])

            # ---- constants / weights ----
            wqkv_sb = consts.tile([128, NT, 3 * C], BF16)
            nc.sync.dma_start(out=wqkv_sb, in_=wqkv_d[:])
            wp_sb = consts.tile([128, NT, C], BF16)
            nc.sync.dma_start(out=wp_sb, in_=wp_d[:])
            qkvb_sb = consts.tile([128, 2 * NT], F32)
            nc.sync.dma_start(out=qkvb_sb, in_=qkvb_d[:])
            vbias_sb = consts.tile([128, NH * 65], BF16)
            nc.sync.dma_start(out=vbias_sb, in_=vbias_d[:])
            pb_sb = consts.tile([128, NT], F32)
            nc.sync.dma_start(out=pb_sb, in_=pb_d[:])
            nw_sb = consts.tile([128, NT], F32)
            nc.sync.dma_start(out=nw_sb, in_=nw_d[:])
            nb_sb = consts.tile([128, NT], F32)
            nc.sync.dma_start(out=nb_sb, in_=nb_d[:])
            gsel_sb = consts.tile([128, 128], F32)
            nc.sync.dma_start(out=gsel_sb, in_=gsel_d[:])
            eps_sb = consts.tile([128, 1], F32)
            nc.vector.memset(eps_sb, EPS)
            ones64 = consts.tile([1, 64], BF16)
            nc.vector.memset(ones64, 1.0)
            warm = consts.tile([1, 1], F32)
            nc.scalar.activation(
                out=warm, in_=eps_sb[0:1, 0:1],
                func=mybir.ActivationFunctionType.Exp, scale=1.0,
            )

            BNS = nc.vector.BN_STATS_DIM   # 6
            BNA = nc.vector.BN_AGGR_DIM    # 2

            # ---- groupnorm (PE+DVE only, no ScalarE): b0 inline, b1 woven in ----
            state = {0: {}, 1: {}}

            def gn_stats_unit(b):
                def u():
                    st = state[b]
                    if b == 0:
                        x_sb = x_first
                    else:
                        x_sb = xpool.tile([128, NT, N], F32, name="x_sb")
                        for t in range(NT):
                            nc.sync.dma_start(out=x_sb[:, t, :], in_=x_d[b, :, t, :])
                    stats4 = spool.tile([128, 2 * NT], F32)
                    for t in range(NT):
                        bnstat = spool.tile([128, 2, BNS], F32)
                        xv = x_sb[:, t, :].rearrange("p (s n) -> p s n", s=2)
                        for s in range(2):
                            nc.vector.bn_stats(out=bnstat[:, s, :], in_=xv[:, s, :])
                        mv = spool.tile([128, BNA], F32)
                        nc.vector.bn_aggr(out=mv, in_=bnstat)
                        nc.vector.tensor_copy(out=stats4[:, t : t + 1], in_=mv[:, 0:1])
                        nc.vector.scalar_tensor_tensor(
                            out=stats4[:, NT + t : NT + t + 1],
                            in0=mv[:, 0:1],
                            scalar=mv[:, 0:1],
                            in1=mv[:, 1:2],
                            op0=mybir.AluOpType.mult,
                            op1=mybir.AluOpType.add,
                        )
                    st["x"] = x_sb
                    st["stats4"] = stats4
                return u

            def gn_finish_unit(b):
                def u():
                    st = state[b]
                    x_sb = st["x"]
                    stats4 = st["stats4"]
                    pst = psum.tile([128, 2 * NT], F32, tag="w", bufs=1)
                    nc.tensor.matmul(pst[:], gsel_sb[:], stats4[:], start=True, stop=True)
                    mean4 = spool.tile([128, NT], F32)
                    nc.vector.tensor_scalar_mul(
                        out=mean4, in0=pst[:, 0:NT], scalar1=1.0 / GS
                    )
                    msq4 = spool.tile([128, NT], F32)
                    nc.vector.tensor_mul(out=msq4, in0=mean4, in1=mean4)
                    var4 = spool.tile([128, NT], F32)
                    nc.vector.scalar_tensor_tensor(
                        out=var4,
                        in0=pst[:, NT : 2 * NT],
                        scalar=1.0 / GS,
                        in1=msq4,
                        op0=mybir.AluOpType.mult,
                        op1=mybir.AluOpType.subtract,
                    )
                    # rstd = 1/sqrt(var + eps): Newton on DVE (ScalarE stays
                    # exp-only so its activation table never swaps)
                    ve = spool.tile([128, NT], F32)
                    nc.vector.tensor_scalar_add(out=ve, in0=var4, scalar1=EPS)
                    vi = ve.bitcast(mybir.dt.int32)
                    sh = spool.tile([128, NT], mybir.dt.int32)
                    nc.vector.tensor_scalar(
                        out=sh, in0=vi, scalar1=1, scalar2=-1,
                        op0=mybir.AluOpType.arith_shift_right,
                        op1=mybir.AluOpType.bitwise_xor,
                    )
                    y0i = spool.tile([128, NT], mybir.dt.int32)
                    nc.vector.tensor_scalar_add(out=y0i, in0=sh, scalar1=0x5F3759E0)
                    rstd4 = y0i.bitcast(F32)
                    for _ in range(2):
                        yy = spool.tile([128, NT], F32)
                        nc.vector.tensor_mul(out=yy, in0=rstd4, in1=rstd4)
                        vyy = spool.tile([128, NT], F32)
                        nc.vector.tensor_mul(out=vyy, in0=ve, in1=yy)
                        w = spool.tile([128, NT], F32)
                        nc.vector.tensor_scalar(
                            out=w, in0=vyy, scalar1=-0.5, scalar2=1.5,
                            op0=mybir.AluOpType.mult, op1=mybir.AluOpType.add,
                        )
                        rs2 = spool.tile([128, NT], F32)
                        nc.vector.tensor_mul(out=rs2, in0=rstd4, in1=w)
                        rstd4 = rs2
                    a4 = spool.tile([128, NT], F32)
                    nc.vector.tensor_mul(out=a4, in0=rstd4, in1=nw_sb)
                    mb4 = spool.tile([128, NT], F32)
                    nc.vector.tensor_mul(out=mb4, in0=mean4, in1=a4)
                    b4 = spool.tile([128, NT], F32)
                    nc.vector.tensor_sub(out=b4, in0=nb_sb, in1=mb4)
                    hn = hnpool.tile([128, NT, N], BF16)
                    for t in range(NT):
                        nc.vector.tensor_scalar(
                            out=hn[:, t, :],
                            in0=x_sb[:, t, :],
                            scalar1=a4[:, t : t + 1],
                            scalar2=b4[:, t : t + 1],
                            op0=mybir.AluOpType.mult,
                            op1=mybir.AluOpType.add,
                        )
                    st["hn"] = hn
                return u

            gn_stats_unit(0)()
            gn_finish_unit(0)()

            # ---- emission helpers (PE queue is in-order: keep ScalarE fed) ----
            def emit_qk_chunk(st, j):
                """q chunk j and k chunk j of qkT (rows j*128 / (NT+j)*128)."""
                hn = st["hn"]
                for rb in (j, NT + j):
                    ps = psum.tile([128, N], F32, tag="w", bufs=1, name="ps_qk")
                    for half in range(2):
                        for kc in range(NT):
                            nc.tensor.matmul(
                                ps[:, half * 512 : (half + 1) * 512],
                                wqkv_sb[:, kc, rb * 128 : (rb + 1) * 128],
                                hn[:, kc, half * 512 : (half + 1) * 512],
                                start=(kc == 0),
                                stop=(kc == NT - 1),
                            )
                    nc.vector.tensor_scalar_add(
                        out=st["qkT"][:, rb, :], in0=ps[:],
                        scalar1=qkvb_sb[:, rb : rb + 1],
                    )

            def emit_proj_rb(st, b, rb, tag="w", bufs=1):
                o_sb = st["o_sb"]
                pp = psum.tile([128, N], F32, tag=tag, bufs=bufs, name="pp")
                for half in range(2):
                    for kc in range(NT):
                        nc.tensor.matmul(
                            pp[:, half * 512 : (half + 1) * 512],
                            wp_sb[:, kc, rb * 128 : (rb + 1) * 128],
                            o_sb[:, kc, half * 512 : (half + 1) * 512],
                            start=(kc == 0),
                            stop=(kc == NT - 1),
                        )
                rx = rxpool.tile([128, N], F32, name="rx")
                nc.sync.dma_start(out=rx, in_=x_d[b, :, rb, :])
                out_sb = outpool.tile([128, N], F32, name="out_sb")
                nc.vector.scalar_tensor_tensor(
                    out=out_sb,
                    in0=pp[:],
                    scalar=pb_sb[:, rb : rb + 1],
                    in1=rx[:],
                    op0=mybir.AluOpType.add,
                    op1=mybir.AluOpType.add,
                )
                nc.sync.dma_start(out=out_d[b, :, rb, :], in_=out_sb[:])

            def emit_scores_unit(st, j, mb, pair):
                """2 concurrent K=64 score matmuls + 2 exps for head pair."""
                pss = [
                    psum.tile([128, N], F32, tag="sc", bufs=2, name="ps_s")
                    for _ in range(2)
                ]
                for half in range(2):
                    for i in range(2):
                        qT_h, kT_h, _ = pair[i]
                        nc.tensor.matmul(
                            pss[i][:, half * 512 : (half + 1) * 512],
                            kT_h[:, mb * 128 : (mb + 1) * 128],
                            qT_h[:, half * 512 : (half + 1) * 512],
                            start=True,
                            stop=True,
                            tile_position=(i * 64, 0),
                        )
                for i in range(2):
                    nc.scalar.activation(
                        out=pair[i][2][:, mb, :], in_=pss[i][:],
                        func=mybir.ActivationFunctionType.Exp, scale=SCALE,
                    )

            def make_pair(st, j):
                qkT = st["qkT"]
                pair = []
                for i in range(2):
                    h = 2 * j + i
                    poff = (h % 2) * 64
                    qT_h = qkT[poff : poff + 64, h // 2, :]
                    kT_h = qkT[poff : poff + 64, NT + h // 2, :]
                    expT = epool.tile([128, 8, N], BF16, name="expT", tag=f"expT{i}")
                    pair.append((qT_h, kT_h, expT))
                return pair

            def av_units(st, j, pair, po_tags=("o", "o")):
                """AV + normalize for pair j as a list of small PE/DVE units."""
                v_pad = st["v_pad"]
                o_sb = st["o_sb"]
                units = []
                for i in range(2):
                    h = 2 * j + i
                    poff = (h % 2) * 64
                    expT = pair[i][2]
                    box = {}
                    po_tag = po_tags[i]

                    def chain(h=h, expT=expT, half=0, box=box, po_tag=po_tag):
                        po = psum.tile([65, 512], F32, tag=po_tag, bufs=2, name="po")
                        for mb in range(8):
                            nc.tensor.matmul(
                                po[:],
                                v_pad[:, mb, h * 65 : (h + 1) * 65],
                                expT[:, mb, half * 512 : (half + 1) * 512],
                                start=(mb == 0),
                                stop=(mb == 7),
                            )
                        denb = dpool.tile([1, 512], BF16, name="denb")
                        with nc.allow_low_precision(reason="softmax denom bf16"):
                            nc.vector.tensor_copy(out=denb, in_=po[64:65, :])
                        box[("po", half)] = po
                        box[("denb", half)] = denb

                    def finish(h=h, poff=poff, box=box):
                        pbc = psum.tile([64, N], F32, tag="w", bufs=1, name="pbc")
                        for half in range(2):
                            nc.tensor.matmul(
                                pbc[:, half * 512 : (half + 1) * 512],
                                ones64[:],
                                box[("denb", half)][:],
                                start=True,
                                stop=True,
                            )
                        recip_bc = rbpool.tile([64, N], F32, name="recip_bc")
                        nc.vector.reciprocal_approx_fast(out=recip_bc, in_=pbc[:])
                        for half in range(2):
                            nc.vector.tensor_mul(
                                out=o_sb[
                                    poff : poff + 64, h // 2,
                                    half * 512 : (half + 1) * 512,
                                ],
                                in0=box[("po", half)][0:64, :],
                                in1=recip_bc[:, half * 512 : (half + 1) * 512],
                            )

                    units.append(lambda c=chain: c(half=0))
                    units.append(lambda c=chain: c(half=1))
                    units.append(finish)
                return units

            def qk_units(st, j, tag="w", bufs=1):
                us = []
                for rb in (j, NT + j):
                    def u(st=st, rb=rb):
                        hn = st["hn"]
                        ps = psum.tile([128, N], F32, tag=tag, bufs=bufs, name="ps_qk")
                        for half in range(2):
                            for kc in range(NT):
                                nc.tensor.matmul(
                                    ps[:, half * 512 : (half + 1) * 512],
                                    wqkv_sb[:, kc, rb * 128 : (rb + 1) * 128],
                                    hn[:, kc, half * 512 : (half + 1) * 512],
                                    start=(kc == 0),
                                    stop=(kc == NT - 1),
                                )
                        nc.vector.tensor_scalar_add(
                            out=st["qkT"][:, rb, :], in0=ps[:],
                            scalar1=qkvb_sb[:, rb : rb + 1],
                        )
                    us.append(u)
                return us

            def v_unit(st, mbp):
                def u(st=st, mbp=mbp):
                    hn = st["hn"]
                    v_pad = st["v_pad"]
                    psv = psum.tile([128, N], F32, tag="w", bufs=1, name="psv")
                    for half in range(2):
                        mb = 2 * mbp + half
                        for kc in range(NT):
                            nc.tensor.matmul(
                                psv[:, half * 512 : (half + 1) * 512],
                                hn[:, kc, mb * 128 : (mb + 1) * 128],
                                wqkv_sb[:, kc, 2 * C : 3 * C],
                                start=(kc == 0),
                                stop=(kc == NT - 1),
                            )
                        nc.vector.tensor_tensor(
                            out=v_pad[:, mb, :].rearrange("p (h c) -> p h c", c=65)[
                                :, :, 0:64
                            ],
                            in0=psv[:, half * 512 : (half + 1) * 512].rearrange(
                                "p (h c) -> p h c", c=64
                            ),
                            in1=vbias_sb.rearrange("p (h c) -> p h c", c=65)[
                                :, :, 0:64
                            ],
                            op=mybir.AluOpType.add,
                        )
                return u

            def proj_unit(st, b, rb):
                def u():
                    emit_proj_rb(st, b, rb)
                return u

            def setup_batch(b):
                st = state[b]
                st["qkT"] = qkpool.tile([128, 2 * NT, N], BF16, name="qkT")
                st["v_pad"] = vpool.tile([128, 8, NH * 65], BF16, name="v_pad")
                ones_view = st["v_pad"].rearrange("p m (h c) -> p m h c", c=65)[
                    :, :, :, 64:65
                ]
                nc.vector.memset(ones_view, 1.0)
                st["o_sb"] = opool.tile([128, NT, N], BF16, name="o_sb")

            setup_batch(0)
            for u in qk_units(state[0], 0, tag="sc", bufs=2):
                u()

            def fillers(b, j):
                s0, s1 = state[0], state.get(1)
                table = {
                    (0, 0): [v_unit(s0, 0), v_unit(s0, 1), v_unit(s0, 2),
                             v_unit(s0, 3)] + qk_units(s0, 1)
                            + [gn_stats_unit(1)],
                    (0, 1): qk_units(s0, 2) + [gn_finish_unit(1)],
                    (0, 2): qk_units(s0, 3),
                    (0, 3): qk_units(s1, 0) + [v_unit(s1, 0), v_unit(s1, 1)],
                    (1, 0): qk_units(s1, 1) + [v_unit(s1, 2), v_unit(s1, 3)],
                    (1, 1): qk_units(s1, 2) + [proj_unit(s0, 0, 0),
                                               proj_unit(s0, 0, 1)],
                    (1, 2): qk_units(s1, 3) + [proj_unit(s0, 0, 2),
                                               proj_unit(s0, 0, 3)],
                    (1, 3): [],
                }
                return table[(b, j)]

            pending = None
            for b in range(BPC):
                st = state[b]
                for j in range(4):
                    if (b, j) == (0, 3):
                        setup_batch(1)
                    pair = make_pair(st, j)
                    av = av_units(*pending) if pending is not None else []
                    fil = fillers(b, j)
                    units = []
                    while av or fil:
                        if av:
                            units.append(av.pop(0))
                        if fil:
                            units.append(fil.pop(0))
                    pending = (st, j, pair)
                    k = 0
                    for mb in range(8):
                        emit_scores_unit(st, j, mb, pair)
                        target = (mb + 1) * len(units) // 8
                        while k < target:
                            units[k]()
                            k += 1

            for u in av_units(*pending, po_tags=("o", "sc")):
                u()
            for rb in range(NT):
                emit_proj_rb(state[BPC - 1], BPC - 1, rb, tag="sc", bufs=2)

    nc.finalize()
    return nc


_PROGRAM = None


def _get_program():
    global _PROGRAM
    if _PROGRAM is None:
        _PROGRAM = build_program()
    return _PROGRAM


def _prep_inputs(x, norm_w, norm_b, qkv_w, qkv_b, proj_w, proj_b):
    x = np.asarray(x, np.float32)
    xs = np.ascontiguousarray(
        x.reshape(B, NT, 128, N).transpose(0, 2, 1, 3)
    )  # (B, 128, NT, N)

    wqkvT = np.asarray(qkv_w, np.float32).T  # (C, 3C)
    wqkv = np.ascontiguousarray(
        wqkvT.reshape(NT, 128, 3 * C).transpose(1, 0, 2)
    ).astype(ml_dtypes.bfloat16)
    wpT = np.asarray(proj_w, np.float32).T
    wp = np.ascontiguousarray(wpT.reshape(NT, 128, C).transpose(1, 0, 2)).astype(
        ml_dtypes.bfloat16
    )

    qkv_b = np.asarray(qkv_b, np.float32)
    qkvb8 = np.ascontiguousarray(qkv_b[: 2 * C].reshape(2 * NT, 128).T)  # (128, 8)
    vb = np.zeros((NH, 65), np.float32)
    vb[:, :64] = qkv_b[2 * C :].reshape(NH, 64)
    vbias = np.ascontiguousarray(
        np.broadcast_to(vb.reshape(1, NH * 65), (128, NH * 65))
    )
    pb4 = np.ascontiguousarray(np.asarray(proj_b, np.float32).reshape(NT, 128).T)
    nw4 = np.ascontiguousarray(np.asarray(norm_w, np.float32).reshape(NT, 128).T)
    nb4 = np.ascontiguousarray(np.asarray(norm_b, np.float32).reshape(NT, 128).T)

    idx = np.arange(128)
    gsel = (idx[:, None] // GS == idx[None, :] // GS).astype(np.float32)

    shared = {
        "wqkv": wqkv, "wp": wp, "qkvb": qkvb8, "vbias": vbias, "pb": pb4,
        "nw": nw4, "nb": nb4, "gsel": gsel,
    }
    in_maps = [
        {"x": np.ascontiguousarray(xs[c * BPC : (c + 1) * BPC]), **shared}
        for c in range(NCORES)
    ]
    return in_maps


def _assemble(results):
    outs = np.concatenate(
        [results[c]["out"] for c in range(NCORES)], axis=0
    )  # (B, 128, NT, N)
    return np.ascontiguousarray(
        outs.transpose(0, 2, 1, 3).reshape(B, C, HH, WW)
    ).astype(np.float32)


def kernel(x, norm_w, norm_b, qkv_w, qkv_b, proj_w, proj_b, _trace=False):
    from concourse.bass_utils import run_bass_kernel_spmd

    nc = _get_program()
    in_maps = _prep_inputs(x, norm_w, norm_b, qkv_w, qkv_b, proj_w, proj_b)
    res = run_bass_kernel_spmd(nc, in_maps, list(range(NCORES)), trace=_trace)
    out = _assemble(res.results)
    if _trace:
        return out, res
    return out


# revision 37
# speedup vs baseline: 1.4159x; 1.0018x over previous
"""AttentionBlock (GroupNorm + 8-head self-attention + proj + residual) on 8 trn2 cores.

Sharding: data-parallel over batch (16 batches -> 2 per core), no collectives.

Per-core device program (per batch):
  - GroupNorm(32, 512): bn_stats per 128-channel tile -> per-channel [mean, E[x^2]]
    -> cross-partition group reduce via a (128,128) group-indicator fp32 matmul
    -> per-channel scale/bias -> hn (bf16).
  - QKV 1x1 conv as matmuls (bf16): q,k produced in (channel, pixel) layout;
    v produced in (pixel, channel) layout, padded with a ones column per head.
  - Attention head-pair j=(2j, 2j+1): the two heads sit at partition offsets
    0/64 of the same qkT chunk, so their K=64 score matmuls occupy disjoint
    PE row-groups (tile_position auto-derived from base partition) and run
    concurrently. exp on ScalarE straight from PSUM (scores bounded ~6.5 ->
    no max subtraction); ScalarE is the kernel bottleneck, so qkv/v/proj
    matmul work is interleaved between score blocks to keep it fed.
  - AV matmul with the ones column producing the softmax denominator as psum
    row 64 (two 1-bank halves for pipelining). Denominator reciprocal ->
    broadcast over 64 partitions via a K=1 ones matmul -> normalize o.
  - proj matmul + (bias + residual) fused in one DVE op -> DMA out.
"""

import numpy as np
import ml_dtypes

import concourse.bass as bass
import concourse.tile as tile
from concourse import bacc, mybir

B, C, HH, WW = 16, 512, 32, 32
N = HH * WW          # 1024 pixels
NH, HD = 8, 64       # heads, head dim
NG, GS = 32, 16      # groups, channels per group
NCORES = 8
BPC = B // NCORES    # batches per core
NT = C // 128        # channel tiles of 128
EPS = 1e-5
SCALE = HD ** -0.5

F32 = mybir.dt.float32
BF16 = mybir.dt.bfloat16


def build_program(qk_bufs=1, out_bufs=2):
    nc = bacc.Bacc(None, target_bir_lowering=False, debug=False)

    x_d = nc.declare_dram_parameter("x", [BPC, 128, NT, N], F32, isOutput=False)
    wqkv_d = nc.declare_dram_parameter("wqkv", [128, NT, 3 * C], BF16, isOutput=False)
    wp_d = nc.declare_dram_parameter("wp", [128, NT, C], BF16, isOutput=False)
    qkvb_d = nc.declare_dram_parameter("qkvb", [128, 2 * NT], F32, isOutput=False)
    vbias_d = nc.declare_dram_parameter("vbias", [128, NH * 65], BF16, isOutput=False)
    pb_d = nc.declare_dram_parameter("pb", [128, NT], F32, isOutput=False)
    nw_d = nc.declare_dram_parameter("nw", [128, NT], F32, isOutput=False)
    nb_d = nc.declare_dram_parameter("nb", [128, NT], F32, isOutput=False)
    gsel_d = nc.declare_dram_parameter("gsel", [128, 128], F32, isOutput=False)
    out_d = nc.declare_dram_parameter("out", [BPC, 128, NT, N], F32, isOutput=True)

    with tile.TileContext(nc) as tc:
        with (
            tc.tile_pool(name="consts", bufs=1) as consts,
            tc.tile_pool(name="xpool", bufs=1) as xpool,
            tc.tile_pool(name="rxpool", bufs=2) as rxpool,
            tc.tile_pool(name="rbpool", bufs=2) as rbpool,
            tc.tile_pool(name="hnpool", bufs=2) as hnpool,
            tc.tile_pool(name="qkpool", bufs=qk_bufs) as qkpool,
            tc.tile_pool(name="vpool", bufs=2) as vpool,
            tc.tile_pool(name="epool", bufs=2) as epool,
            tc.tile_pool(name="opool", bufs=2) as opool,
            tc.tile_pool(name="dpool", bufs=4) as dpool,
            tc.tile_pool(name="outpool", bufs=out_bufs) as outpool,
            tc.tile_pool(name="spool", bufs=2) as spool,
            tc.tile_pool(name="psum", bufs=2, space="PSUM") as psum,
        ):
            # ---- x for batch 0 first: it gates the whole pipeline ----
            x_first = xpool.tile([128, NT, N], F32, name="x_sb")
            for t in range(NT):
                nc.sync.dma_start(out=x_first[:, t, :], in_=x_d[0, :, t, :])

            # ---- constants / weights ----
            wqkv_sb = consts.tile([128, NT, 3 * C], BF16)
            nc.sync.dma_start(out=wqkv_sb, in_=wqkv_d[:])
            wp_sb = consts.tile([128, NT, C], BF16)
            nc.sync.dma_start(out=wp_sb, in_=wp_d[:])
            qkvb_sb = consts.tile([128, 2 * NT], F32)
            nc.sync.dma_start(out=qkvb_sb, in_=qkvb_d[:])
            vbias_sb = consts.tile([128, NH * 65], BF16)
            nc.sync.dma_start(out=vbias_sb, in_=vbias_d[:])
            pb_sb = consts.tile([128, NT], F32)
            nc.sync.dma_start(out=pb_sb, in_=pb_d[:])
            nw_sb = consts.tile([128, NT], F32)
            nc.sync.dma_start(out=nw_sb, in_=nw_d[:])
            nb_sb = consts.tile([128, NT], F32)
            nc.sync.dma_start(out=nb_sb, in_=nb_d[:])
            gsel_sb = consts.tile([128, 128], F32)
            nc.sync.dma_start(out=gsel_sb, in_=gsel_d[:])
            eps_sb = consts.tile([128, 1], F32)
            nc.vector.memset(eps_sb, EPS)
            ones64 = consts.tile([1, 64], BF16)
            nc.vector.memset(ones64, 1.0)
            warm = consts.tile([1, 1], F32)
            nc.scalar.activation(
                out=warm, in_=eps_sb[0:1, 0:1],
                func=mybir.ActivationFunctionType.Exp, scale=1.0,
            )

            BNS = nc.vector.BN_STATS_DIM   # 6
            BNA = nc.vector.BN_AGGR_DIM    # 2

            # ---- groupnorm for both batches (all sqrt ACT ops before any exp) ----
            state = {}
            for b in range(BPC):
                if b == 0:
                    x_sb = x_first
                else:
                    x_sb = xpool.tile([128, NT, N], F32, name="x_sb")
                    for t in range(NT):
                        nc.sync.dma_start(out=x_sb[:, t, :], in_=x_d[b, :, t, :])

                stats4 = spool.tile([128, 2 * NT], F32)
                for t in range(NT):
                    bnstat = spool.tile([128, 2, BNS], F32)
                    xv = x_sb[:, t, :].rearrange("p (s n) -> p s n", s=2)
                    for s in range(2):
                        nc.vector.bn_stats(out=bnstat[:, s, :], in_=xv[:, s, :])
                    mv = spool.tile([128, BNA], F32)
                    nc.vector.bn_aggr(out=mv, in_=bnstat)
                    nc.vector.tensor_copy(out=stats4[:, t : t + 1], in_=mv[:, 0:1])
                    nc.vector.scalar_tensor_tensor(
                        out=stats4[:, NT + t : NT + t + 1],
                        in0=mv[:, 0:1],
                        scalar=mv[:, 0:1],
                        in1=mv[:, 1:2],
                        op0=mybir.AluOpType.mult,
                        op1=mybir.AluOpType.add,
                    )

                pst = psum.tile([128, 2 * NT], F32, tag="w", bufs=1)
                nc.tensor.matmul(pst[:], gsel_sb[:], stats4[:], start=True, stop=True)

                mean4 = spool.tile([128, NT], F32)
                nc.vector.tensor_scalar_mul(out=mean4, in0=pst[:, 0:NT], scalar1=1.0 / GS)
                msq4 = spool.tile([128, NT], F32)
                nc.vector.tensor_mul(out=msq4, in0=mean4, in1=mean4)
                var4 = spool.tile([128, NT], F32)
                nc.vector.scalar_tensor_tensor(
                    out=var4,
                    in0=pst[:, NT : 2 * NT],
                    scalar=1.0 / GS,
                    in1=msq4,
                    op0=mybir.AluOpType.mult,
                    op1=mybir.AluOpType.subtract,
                )
                # rstd = 1/sqrt(var + eps), Newton on DVE (keeps ScalarE
                # exp-only so its activation table never swaps)
                ve = spool.tile([128, NT], F32)
                nc.vector.tensor_scalar_add(out=ve, in0=var4, scalar1=EPS)
                vi = ve.bitcast(mybir.dt.int32)
                sh = spool.tile([128, NT], mybir.dt.int32)
                nc.vector.tensor_scalar(
                    out=sh, in0=vi, scalar1=1, scalar2=-1,
                    op0=mybir.AluOpType.arith_shift_right,
                    op1=mybir.AluOpType.bitwise_xor,
                )
                y0i = spool.tile([128, NT], mybir.dt.int32)
                nc.vector.tensor_scalar_add(out=y0i, in0=sh, scalar1=0x5F3759E0)
                rstd4 = y0i.bitcast(F32)
                for _ in range(2):
                    yy = spool.tile([128, NT], F32)
                    nc.vector.tensor_mul(out=yy, in0=rstd4, in1=rstd4)
                    vyy = spool.tile([128, NT], F32)
                    nc.vector.tensor_mul(out=vyy, in0=ve, in1=yy)
                    w = spool.tile([128, NT], F32)
                    nc.vector.tensor_scalar(
                        out=w, in0=vyy, scalar1=-0.5, scalar2=1.5,
                        op0=mybir.AluOpType.mult, op1=mybir.AluOpType.add,
                    )
                    rs2 = spool.tile([128, NT], F32)
                    nc.vector.tensor_mul(out=rs2, in0=rstd4, in1=w)
                    rstd4 = rs2
                a4 = spool.tile([128, NT], F32)
                nc.vector.tensor_mul(out=a4, in0=rstd4, in1=nw_sb)
                mb4 = spool.tile([128, NT], F32)
                nc.vector.tensor_mul(out=mb4, in0=mean4, in1=a4)
                b4 = spool.tile([128, NT], F32)
                nc.vector.tensor_sub(out=b4, in0=nb_sb, in1=mb4)

                hn = hnpool.tile([128, NT, N], BF16)
                for t in range(NT):
                    nc.vector.tensor_scalar(
                        out=hn[:, t, :],
                        in0=x_sb[:, t, :],
                        scalar1=a4[:, t : t + 1],
                        scalar2=b4[:, t : t + 1],
                        op0=mybir.AluOpType.mult,
                        op1=mybir.AluOpType.add,
                    )
                state[b] = {"x": x_sb, "hn": hn}

            # ---- emission helpers (PE queue is in-order: keep ScalarE fed) ----
            def emit_qk_chunk(st, j):
                """q chunk j and k chunk j of qkT (rows j*128 / (NT+j)*128)."""
                hn = st["hn"]
                for rb in (j, NT + j):
                    ps = psum.tile([128, N], F32, tag="w", bufs=1, name="ps_qk")
                    for half in range(2):
                        for kc in range(NT):
                            nc.tensor.matmul(
                                ps[:, half * 512 : (half + 1) * 512],
                                wqkv_sb[:, kc, rb * 128 : (rb + 1) * 128],
                                hn[:, kc, half * 512 : (half + 1) * 512],
                                start=(kc == 0),
                                stop=(kc == NT - 1),
                            )
                    nc.vector.tensor_scalar_add(
                        out=st["qkT"][:, rb, :], in0=ps[:],
                        scalar1=qkvb_sb[:, rb : rb + 1],
                    )

            def emit_proj_rb(st, b, rb, tag="w", bufs=1):
                o_sb = st["o_sb"]
                pp = psum.tile([128, N], F32, tag=tag, bufs=bufs, name="pp")
                for half in range(2):
                    for kc in range(NT):
                        nc.tensor.matmul(
                            pp[:, half * 512 : (half + 1) * 512],
                            wp_sb[:, kc, rb * 128 : (rb + 1) * 128],
                            o_sb[:, kc, half * 512 : (half + 1) * 512],
                            start=(kc == 0),
                            stop=(kc == NT - 1),
                        )
                rx = rxpool.tile([128, N], F32, name="rx")
                nc.sync.dma_start(out=rx, in_=x_d[b, :, rb, :])
                out_sb = outpool.tile([128, N], F32, name="out_sb")
                nc.vector.scalar_tensor_tensor(
                    out=out_sb,
                    in0=pp[:],
                    scalar=pb_sb[:, rb : rb + 1],
                    in1=rx[:],
                    op0=mybir.AluOpType.add,
                    op1=mybir.AluOpType.add,
                )
                nc.sync.dma_start(out=out_d[b, :, rb, :], in_=out_sb[:])

            def emit_scores_unit(st, j, mb, pair):
                """2 concurrent K=64 score matmuls + 2 exps for head pair."""
                pss = [
                    psum.tile([128, N], F32, tag="sc", bufs=2, name="ps_s")
                    for _ in range(2)
                ]
                for half in range(2):
                    for i in range(2):
                        qT_h, kT_h, _ = pair[i]
                        nc.tensor.matmul(
                            pss[i][:, half * 512 : (half + 1) * 512],
                            kT_h[:, mb * 128 : (mb + 1) * 128],
                            qT_h[:, half * 512 : (half + 1) * 512],
                            start=True,
                            stop=True,
                            tile_position=(i * 64, 0),
                        )
                for i in range(2):
                    nc.scalar.activation(
                        out=pair[i][2][:, mb, :], in_=pss[i][:],
                        func=mybir.ActivationFunctionType.Exp, scale=SCALE,
                    )

            def make_pair(st, j):
                qkT = st["qkT"]
                pair = []
                for i in range(2):
                    h = 2 * j + i
                    poff = (h % 2) * 64
                    qT_h = qkT[poff : poff + 64, h // 2, :]
                    kT_h = qkT[poff : poff + 64, NT + h // 2, :]
                    expT = epool.tile([128, 8, N], BF16, name="expT", tag=f"expT{i}")
                    pair.append((qT_h, kT_h, expT))
                return pair

            def av_units(st, j, pair, po_tags=("o", "o")):
                """AV + normalize for pair j as a list of small PE/DVE units."""
                v_pad = st["v_pad"]
                o_sb = st["o_sb"]
                units = []
                for i in range(2):
                    h = 2 * j + i
                    poff = (h % 2) * 64
                    expT = pair[i][2]
                    box = {}
                    po_tag = po_tags[i]

                    def chain(h=h, expT=expT, half=0, box=box, po_tag=po_tag):
                        po = psum.tile([65, 512], F32, tag=po_tag, bufs=2, name="po")
                        for mb in range(8):
                            nc.tensor.matmul(
                                po[:],
                                v_pad[:, mb, h * 65 : (h + 1) * 65],
                                expT[:, mb, half * 512 : (half + 1) * 512],
                                start=(mb == 0),
                                stop=(mb == 7),
                            )
                        denb = dpool.tile([1, 512], BF16, name="denb")
                        with nc.allow_low_precision(reason="softmax denom bf16"):
                            nc.vector.tensor_copy(out=denb, in_=po[64:65, :])
                        box[("po", half)] = po
                        box[("denb", half)] = denb

                    def finish(h=h, poff=poff, box=box):
                        pbc = psum.tile([64, N], F32, tag="w", bufs=1, name="pbc")
                        for half in range(2):
                            nc.tensor.matmul(
                                pbc[:, half * 512 : (half + 1) * 512],
                                ones64[:],
                                box[("denb", half)][:],
                                start=True,
                                stop=True,
                            )
                        recip_bc = rbpool.tile([64, N], F32, name="recip_bc")
                        nc.vector.reciprocal_approx_fast(out=recip_bc, in_=pbc[:])
                        for half in range(2):
                            nc.vector.tensor_mul(
                                out=o_sb[
                                    poff : poff + 64, h // 2,
                                    half * 512 : (half + 1) * 512,
                                ],
                                in0=box[("po", half)][0:64, :],
                                in1=recip_bc[:, half * 512 : (half + 1) * 512],
                            )

                    units.append(lambda c=chain: c(half=0))
                    units.append(lambda c=chain: c(half=1))
                    units.append(finish)
                return units

            def qk_units(st, j, tag="w", bufs=1):
                us = []
                for rb in (j, NT + j):
                    def u(st=st, rb=rb):
                        hn = st["hn"]
                        ps = psum.tile([128, N], F32, tag=tag, bufs=bufs, name="ps_qk")
                        for half in range(2):
                            for kc in range(NT):
                                nc.tensor.matmul(
                                    ps[:, half * 512 : (half + 1) * 512],
                                    wqkv_sb[:, kc, rb * 128 : (rb + 1) * 128],
                                    hn[:, kc, half * 512 : (half + 1) * 512],
                                    start=(kc == 0),
                                    stop=(kc == NT - 1),
                                )
                        nc.vector.tensor_scalar_add(
                            out=st["qkT"][:, rb, :], in0=ps[:],
                            scalar1=qkvb_sb[:, rb : rb + 1],
                        )
                    us.append(u)
                return us

            def v_unit(st, mbp):
                def u(st=st, mbp=mbp):
                    hn = st["hn"]
                    v_pad = st["v_pad"]
                    psv = psum.tile([128, N], F32, tag="w", bufs=1, name="psv")
                    for half in range(2):
                        mb = 2 * mbp + half
                        for kc in range(NT):
                            nc.tensor.matmul(
                                psv[:, half * 512 : (half + 1) * 512],
                                hn[:, kc, mb * 128 : (mb + 1) * 128],
                                wqkv_sb[:, kc, 2 * C : 3 * C],
                                start=(kc == 0),
                                stop=(kc == NT - 1),
                            )
                        nc.vector.tensor_tensor(
                            out=v_pad[:, mb, :].rearrange("p (h c) -> p h c", c=65)[
                                :, :, 0:64
                            ],
                            in0=psv[:, half * 512 : (half + 1) * 512].rearrange(
                                "p (h c) -> p h c", c=64
                            ),
                            in1=vbias_sb.rearrange("p (h c) -> p h c", c=65)[
                                :, :, 0:64
                            ],
                            op=mybir.AluOpType.add,
                        )
                return u

            def proj_unit(st, b, rb):
                def u():
                    emit_proj_rb(st, b, rb)
                return u

            def setup_batch(b):
                st = state[b]
                st["qkT"] = qkpool.tile([128, 2 * NT, N], BF16, name="qkT")
                st["v_pad"] = vpool.tile([128, 8, NH * 65], BF16, name="v_pad")
                ones_view = st["v_pad"].rearrange("p m (h c) -> p m h c", c=65)[
                    :, :, :, 64:65
                ]
                nc.vector.memset(ones_view, 1.0)
                st["o_sb"] = opool.tile([128, NT, N], BF16, name="o_sb")

            setup_batch(0)
            for u in qk_units(state[0], 0, tag="sc", bufs=2):
                u()

            def fillers(b, j):
                s0, s1 = state[0], state.get(1)
                table = {
                    (0, 0): [v_unit(s0, 0), v_unit(s0, 1), v_unit(s0, 2),
                             v_unit(s0, 3)] + qk_units(s0, 1),
                    (0, 1): qk_units(s0, 2),
                    (0, 2): qk_units(s0, 3),
                    (0, 3): qk_units(s1, 0) + [v_unit(s1, 0), v_unit(s1, 1)],
                    (1, 0): qk_units(s1, 1) + [v_unit(s1, 2), v_unit(s1, 3)],
                    (1, 1): qk_units(s1, 2) + [proj_unit(s0, 0, 0),
                                               proj_unit(s0, 0, 1)],
                    (1, 2): qk_units(s1, 3) + [proj_unit(s0, 0, 2),
                                               proj_unit(s0, 0, 3)],
                    (1, 3): [],
                }
                return table[(b, j)]

            pending = None
            for b in range(BPC):
                st = state[b]
                for j in range(4):
                    if (b, j) == (0, 3):
                        setup_batch(1)
                    pair = make_pair(st, j)
                    av = av_units(*pending) if pending is not None else []
                    fil = fillers(b, j)
                    units = []
                    while av or fil:
                        if av:
                            units.append(av.pop(0))
                        if fil:
                            units.append(fil.pop(0))
                    pending = (st, j, pair)
                    k = 0
                    for mb in range(8):
                        emit_scores_unit(st, j, mb, pair)
                        target = (mb + 1) * len(units) // 8
                        while k < target:
                            units[k]()
                            k += 1

            for u in av_units(*pending, po_tags=("o", "sc")):
                u()
            for rb in range(NT):
                emit_proj_rb(state[BPC - 1], BPC - 1, rb, tag="sc", bufs=2)

    nc.finalize()
    return nc


_PROGRAM = None


def _get_program():
    global _PROGRAM
    if _PROGRAM is None:
        _PROGRAM = build_program()
    return _PROGRAM


def _prep_inputs(x, norm_w, norm_b, qkv_w, qkv_b, proj_w, proj_b):
    x = np.asarray(x, np.float32)
    xs = np.ascontiguousarray(
        x.reshape(B, NT, 128, N).transpose(0, 2, 1, 3)
    )  # (B, 128, NT, N)

    wqkvT = np.asarray(qkv_w, np.float32).T  # (C, 3C)
    wqkv = np.ascontiguousarray(
        wqkvT.reshape(NT, 128, 3 * C).transpose(1, 0, 2)
    ).astype(ml_dtypes.bfloat16)
    wpT = np.asarray(proj_w, np.float32).T
    wp = np.ascontiguousarray(wpT.reshape(NT, 128, C).transpose(1, 0, 2)).astype(
        ml_dtypes.bfloat16
    )

    qkv_b = np.asarray(qkv_b, np.float32)
    qkvb8 = np.ascontiguousarray(qkv_b[: 2 * C].reshape(2 * NT, 128).T)  # (128, 8)
    vb = np.zeros((NH, 65), np.float32)
    vb[:, :64] = qkv_b[2 * C :].reshape(NH, 64)
    vbias = np.ascontiguousarray(
        np.broadcast_to(vb.reshape(1, NH * 65), (128, NH * 65))
    )
    pb4 = np.ascontiguousarray(np.asarray(proj_b, np.float32).reshape(NT, 128).T)
    nw4 = np.ascontiguousarray(np.asarray(norm_w, np.float32).reshape(NT, 128).T)
    nb4 = np.ascontiguousarray(np.asarray(norm_b, np.float32).reshape(NT, 128).T)

    idx = np.arange(128)
    gsel = (idx[:, None] // GS == idx[None, :] // GS).astype(np.float32)

    shared = {
        "wqkv": wqkv, "wp": wp, "qkvb": qkvb8, "vbias": vbias, "pb": pb4,
        "nw": nw4, "nb": nb4, "gsel": gsel,
    }
    in_maps = [
        {"x": np.ascontiguousarray(xs[c * BPC : (c + 1) * BPC]), **shared}
        for c in range(NCORES)
    ]
    return in_maps


def _assemble(results):
    outs = np.concatenate(
        [results[c]["out"] for c in range(NCORES)], axis=0
    )  # (B, 128, NT, N)
    return np.ascontiguousarray(
        outs.transpose(0, 2, 1, 3).reshape(B, C, HH, WW)
    ).astype(np.float32)


def kernel(x, norm_w, norm_b, qkv_w, qkv_b, proj_w, proj_b, _trace=False):
    from concourse.bass_utils import run_bass_kernel_spmd

    nc = _get_program()
    in_maps = _prep_inputs(x, norm_w, norm_b, qkv_w, qkv_b, proj_w, proj_b)
    res = run_bass_kernel_spmd(nc, in_maps, list(range(NCORES)), trace=_trace)
    out = _assemble(res.results)
    if _trace:
        return out, res
    return out
